# revision 15
# baseline (speedup 1.0000x reference)
"""Trainium2 Bass kernel for nn_BasicVSR_LFN (upflow + backwarp + 7x7
correlation + 4 convs), data-parallel over batch: 1 sample per NeuronCore.

Per-core pipeline (shapes hardcoded for B=8, C=96, H=96, W=160):
  1. upflow (ConvTranspose2d 2->2, k4 s2 p1 groups=2) as 4 parity-plane
     stencils on DVE, assembled to flow[g][96,160] by strided DMA.
  2. warp coordinate/index/weight pipeline in [96y,160x] layout on DVE.
  3. idx -> interleaved int16 [96,960] (PE transpose + strided copies +
     partition-doubling DMA) for gpsimd ap_gather.
  4. backwarp: 4 ap_gather taps from padded f2 (AP offsets 0/1/160/161),
     bilinear combine on DVE; weights replicated across channel partitions
     by log-doubling DMAs per quarter. Output: padded channel-major warped
     bf16 [96, 102, 166].
  5. correlation: 120 PE matmuls (f1 16x8-pixel tile [96,128] x warped
     window [96,22,14]); PSUM [128,308] -> bf16 -> gpsimd local_scatter
     (static band table) -> [128,50] pixel-major -> PE pair transpose ->
     channel-major corr2 [100, 98, 163] (2x col-shift K-stack), lrelu.
  6. convs as K-packed shifted matmuls (N=480 row-aligned chunks), PSUM
     accumulation, fused bias+leaky-relu+bf16 cast via ACT Prelu copy-out.
     conv4 5x5 via 4-row-shift K-stack; out = flow + res.
"""
import numpy as np
import ml_dtypes

import concourse.bass as bass
import concourse.bacc as bacc
import concourse.mybir as mybir
import concourse.tile as tile
from contextlib import ExitStack

F32 = mybir.dt.float32
BF16 = mybir.dt.float16  # fp16 everywhere (precision margin)
I16 = mybir.dt.int16
I32 = mybir.dt.int32
ALU = mybir.AluOpType
ACTF = mybir.ActivationFunctionType

B, C, H, W = 8, 96, 96, 160
HW = H * W
N_CORES = 8
PAD = 161
F2LEN = PAD + HW + PAD         # 15682
NEG = HW + PAD                 # 15521
AX = 2.5 * W / (W - 1.0)
AY = 2.5 * H / (H - 1.0)
EPSF = -0.5 + 2.0 ** -11

WPH, WPW = H + 6, W + 6        # 102, 166
CPH, CPW = H + 2, W + 3        # 98, 163
QPH, QPW = H + 4, W + 5        # 100, 165

GCH = 960
QP = 3840
NCH = 32                       # conv chunks (3 rows x 160)
CHP = 480

bf = np.float16

# upflow tap order (must match host table)
UP_ORDER = [(g, ry, rx, di, dj)
            for g in range(2) for ry in range(2) for rx in range(2)
            for di in ([-1, 0] if ry == 0 else [0, 1])
            for dj in ([-1, 0] if rx == 0 else [0, 1])]


def _host_consts():
    cs = {}
    cs["xg"] = np.tile(np.arange(W, dtype=np.float32)[None, :], (H, 1))
    cs["yg"] = np.tile(np.arange(H, dtype=np.float32)[:, None], (1, W))
    cs["id96"] = np.eye(96, dtype=np.float32)
    cs["id128b"] = np.eye(128, dtype=np.float32).astype(bf)
    lsi = np.full((128, 308), -1, dtype=np.int16)
    for m in range(128):
        r, c = m // 8, m % 8
        for n in range(308):
            Rr, Cc = n // 14, n % 14
            dy, dx = Rr - 3 - r, Cc - 3 - c
            if -3 <= dy <= 3 and -3 <= dx <= 3:
                lsi[m, n] = (dy + 3) * 7 + (dx + 3)
    cs["lsidx"] = lsi
    return cs


def _host_weights(up_w, w1, b1, w2, b2, w3, b3, w4, b4):
    ws = {}
    tab = np.zeros((48, 32), np.float32)
    for j, (g, ry, rx, di, dj) in enumerate(UP_ORDER):
        tab[:, j] = up_w[g, 0, 1 - 2 * di + ry, 1 - 2 * dj + rx]
    ws["upwtab"] = tab
    # conv1 pair [114, 3*128] (kx0 @ rows 0-48, kx1 @ rows 64-112),
    # single [50, 3*128]
    c1p = np.zeros((3, 114, 128), np.float32)
    c1s = np.zeros((3, 50, 128), np.float32)
    for ky in range(3):
        c1p[ky, 0:49] = w1[:, :, ky, 0].T
        c1p[ky, 64:113] = w1[:, :, ky, 1].T
        c1s[ky, 0:49] = w1[:, :, ky, 2].T
    ws["c1p"] = np.transpose(c1p, (1, 0, 2)).reshape(114, 384).astype(bf)
    ws["c1s"] = np.transpose(c1s, (1, 0, 2)).reshape(50, 384).astype(bf)
    c2 = np.zeros((9, 128, 64), np.float32)
    for ky in range(3):
        for kx in range(3):
            c2[ky * 3 + kx] = w2[:, :, ky, kx].T
    ws["c2"] = np.transpose(c2, (1, 0, 2)).reshape(128, 576).astype(bf)
    c3p = np.zeros((3, 128, 32), np.float32)
    c3s = np.zeros((3, 64, 32), np.float32)
    for ky in range(3):
        c3p[ky, 0:64] = w3[:, :, ky, 0].T
        c3p[ky, 64:128] = w3[:, :, ky, 1].T
        c3s[ky] = w3[:, :, ky, 2].T
    ws["c3p"] = np.transpose(c3p, (1, 0, 2)).reshape(128, 96).astype(bf)
    ws["c3s"] = np.transpose(c3s, (1, 0, 2)).reshape(64, 96).astype(bf)
    c4q = np.zeros((5, 128, 2), np.float32)
    c4s = np.zeros((5, 32, 2), np.float32)
    for kx in range(5):
        for dr in range(4):
            c4q[kx, dr * 32:(dr + 1) * 32] = w4[:, :, dr, kx].T
        c4s[kx] = w4[:, :, 4, kx].T
    ws["c4q"] = np.transpose(c4q, (1, 0, 2)).reshape(128, 10).astype(bf)
    ws["c4s"] = np.transpose(c4s, (1, 0, 2)).reshape(32, 10).astype(bf)
    for nm, b_ in (("b1", b1), ("b2", b2), ("b3", b3), ("b4", b4)):
        ws[nm] = np.asarray(b_, np.float32)[:, None]
    ws["al128"] = np.full((128, 1), 0.1, np.float32)
    ws["al64"] = np.full((64, 1), 0.1, np.float32)
    ws["al32"] = np.full((32, 1), 0.1, np.float32)
    return ws


def build_program(debug=False):
    nc = bacc.Bacc("TRN2", target_bir_lowering=False, debug=False,
                   num_devices=N_CORES, num_swdge_queues=4)
    P = nc.declare_dram_parameter
    d_f1b = P("f1b", [96, HW], BF16, isOutput=False)
    d_f2t = P("f2t", [F2LEN, 128], BF16, isOutput=False)
    d_fp = P("fp", [2, 48, 80], F32, isOutput=False)
    d_upw = P("upwtab", [48, 32], F32, isOutput=False)
    d_xg = P("xg", [H, W], F32, isOutput=False)
    d_yg = P("yg", [H, W], F32, isOutput=False)
    d_id96 = P("id96", [96, 96], F32, isOutput=False)
    d_id128b = P("id128b", [128, 128], BF16, isOutput=False)
    d_lsi = P("lsidx", [128, 308], I16, isOutput=False)
    d_c1p = P("c1p", [114, 384], BF16, isOutput=False)
    d_c1s = P("c1s", [50, 384], BF16, isOutput=False)
    d_c2 = P("c2", [128, 576], BF16, isOutput=False)
    d_c3p = P("c3p", [128, 96], BF16, isOutput=False)
    d_c3s = P("c3s", [64, 96], BF16, isOutput=False)
    d_c4q = P("c4q", [128, 10], BF16, isOutput=False)
    d_c4s = P("c4s", [32, 10], BF16, isOutput=False)
    d_b1 = P("b1", [128, 1], F32, isOutput=False)
    d_b2 = P("b2", [64, 1], F32, isOutput=False)
    d_b3 = P("b3", [32, 1], F32, isOutput=False)
    d_b4 = P("b4", [2, 1], F32, isOutput=False)
    d_al128 = P("al128", [128, 1], F32, isOutput=False)
    d_al64 = P("al64", [64, 1], F32, isOutput=False)
    d_al32 = P("al32", [32, 1], F32, isOutput=False)
    d_out = P("out", [2, H, W], F32, isOutput=True)
    d_wsc = nc.dram_tensor("wscratch", [4, HW], BF16)
    dbg = {}
    if debug:
        dbg["warped"] = P("dbg_warped", [96, WPH, WPW], BF16, isOutput=True)
        dbg["corr2"] = P("dbg_corr2", [114, CPH, CPW], BF16, isOutput=True)
        dbg["h1"] = P("dbg_h1", [128, CPH, CPW], BF16, isOutput=True)
        dbg["h3"] = P("dbg_h3", [128, QPH, QPW], BF16, isOutput=True)
        dbg["flow"] = P("dbg_flow", [2, H, W], F32, isOutput=True)
        dbg["il"] = P("dbg_il", [128, 960], I16, isOutput=True)

    with tile.TileContext(nc) as tc, ExitStack() as top:
        pc = top.enter_context(tc.tile_pool(name="pc", bufs=1))

        # ---- consts ----
        t_upw = pc.tile([48, 32], F32)
        t_id96 = pc.tile([96, 96], F32)
        t_id128b = pc.tile([128, 128], BF16)
        t_lsi = pc.tile([128, 308], I16)
        t_c1p = pc.tile([114, 384], BF16)
        t_c1s = pc.tile([50, 384], BF16)
        t_c2 = pc.tile([128, 576], BF16)
        t_c3p = pc.tile([128, 96], BF16)
        t_c3s = pc.tile([64, 96], BF16)
        t_c4q = pc.tile([128, 10], BF16)
        t_c4s = pc.tile([32, 10], BF16)
        t_b1 = pc.tile([128, 1], F32)
        t_b2 = pc.tile([64, 1], F32)
        t_b3 = pc.tile([32, 1], F32)
        t_b4 = pc.tile([2, 1], F32)
        t_al128 = pc.tile([128, 1], F32)
        t_al64 = pc.tile([64, 1], F32)
        t_al32 = pc.tile([32, 1], F32)
        for tt, dd in ((t_upw, d_upw),
                       (t_id96, d_id96), (t_id128b, d_id128b),
                       (t_lsi, d_lsi), (t_c1p, d_c1p), (t_c1s, d_c1s),
                       (t_c2, d_c2), (t_c3p, d_c3p), (t_c3s, d_c3s),
                       (t_c4q, d_c4q), (t_c4s, d_c4s), (t_b1, d_b1),
                       (t_b2, d_b2), (t_b3, d_b3), (t_b4, d_b4),
                       (t_al128, d_al128), (t_al64, d_al64),
                       (t_al32, d_al32)):
            nc.sync.dma_start(tt[:], dd[:])

        # ---- upflow ----
        pp_cm = tc.tile_pool(name="pp", bufs=1)
        pp = pp_cm.__enter__()
        t_xg = pp.tile([H, W], F32)
        t_yg = pp.tile([H, W], F32)
        nc.sync.dma_start(t_xg[:], d_xg[:])
        nc.sync.dma_start(t_yg[:], d_yg[:])
        # fps[g][di+1]: flow_prev[g, p+di, q+dj] readable at col offset dj+1
        fps = {}
        for g in range(2):
            for di in (-1, 0, 1):
                nm = f"fps{g}_{di + 1}"
                t = pp.tile([48, 82], F32, tag=nm, name=nm)
                nc.vector.memset(t[:], 0.0)
                lo, hi = max(0, di), min(48, 48 + di)
                nc.sync.dma_start(t[lo - di:hi - di, 1:81], d_fp[g, lo:hi, :])
                fps[(g, di)] = t
        t_upt = pp.tile([48, 80], F32, tag="uptmp")
        planes = {}
        for key in {(g, ry, rx) for (g, ry, rx, _, _) in UP_ORDER}:
            nm = f"pl{key[0]}{key[1]}{key[2]}"
            planes[key] = pp.tile([48, 80], F32, tag=nm, name=nm)
        done = set()
        for j, (g, ry, rx, di, dj) in enumerate(UP_ORDER):
            pl = planes[(g, ry, rx)]
            sc = t_upw[:, j:j + 1]
            src = fps[(g, di)][:, 1 + dj:81 + dj]
            if (g, ry, rx) not in done:
                done.add((g, ry, rx))
                nc.vector.tensor_scalar(pl[:], src, sc, None, ALU.mult)
            else:
                nc.vector.tensor_scalar(t_upt[:], src, sc, None, ALU.mult)
                nc.vector.tensor_tensor(pl[:], pl[:], t_upt[:], ALU.add)
        t_flx = pc.tile([H, W], F32)
        t_fly = pc.tile([H, W], F32)
        flyx = [t_flx, t_fly]
        for (g, ry, rx), pl in sorted(planes.items()):
            nc.sync.dma_start(flyx[g][ry::2, rx::2], pl[:])

        # ---- warp index / weight pipeline ----
        def hwt(tag, dt=F32):
            return pp.tile([H, W], dt, tag=tag, name=tag)

        t_px, t_py = hwt("px"), hwt("py")
        nc.vector.tensor_scalar(t_px[:], t_flx[:], AX, None, ALU.mult)
        nc.vector.tensor_tensor(t_px[:], t_px[:], t_xg[:], ALU.add)
        nc.vector.tensor_scalar(t_py[:], t_fly[:], AY, None, ALU.mult)
        nc.vector.tensor_tensor(t_py[:], t_py[:], t_yg[:], ALU.add)
        t_x0, t_y0 = hwt("x0"), hwt("y0")
        t_i32 = pp.tile([H, W], I32, tag="i32")
        nc.vector.tensor_scalar(t_x0[:], t_px[:], EPSF, None, ALU.add)
        nc.vector.tensor_copy(t_i32[:], t_x0[:])
        nc.vector.tensor_copy(t_x0[:], t_i32[:])
        nc.vector.tensor_scalar(t_y0[:], t_py[:], EPSF, None, ALU.add)
        nc.vector.tensor_copy(t_i32[:], t_y0[:])
        nc.vector.tensor_copy(t_y0[:], t_i32[:])
        t_wx1, t_wy1, t_wx0, t_wy0 = (hwt("wx1"), hwt("wy1"),
                                      hwt("wx0"), hwt("wy0"))
        nc.vector.tensor_tensor(t_wx1[:], t_px[:], t_x0[:], ALU.subtract)
        nc.vector.tensor_tensor(t_wy1[:], t_py[:], t_y0[:], ALU.subtract)
        nc.vector.tensor_scalar(t_wx0[:], t_wx1[:], -1.0, 1.0, ALU.mult, ALU.add)
        nc.vector.tensor_scalar(t_wy0[:], t_wy1[:], -1.0, 1.0, ALU.mult, ALU.add)
        t_m1, t_m2 = hwt("m1"), hwt("m2")
        wviews = {}
        for nm, t_base, t_w, lo, hi in (
                ("wx0", t_x0, t_wx0, 0.0, float(W - 1)),
                ("wx1", t_x0, t_wx1, -1.0, float(W - 2)),
                ("wy0", t_y0, t_wy0, 0.0, float(H - 1)),
                ("wy1", t_y0, t_wy1, -1.0, float(H - 2))):
            nc.vector.tensor_scalar(t_m1[:], t_base[:], lo, None, ALU.is_ge)
            nc.vector.tensor_scalar(t_m2[:], t_base[:], hi, None, ALU.is_le)
            nc.vector.tensor_tensor(t_m1[:], t_m1[:], t_m2[:], ALU.mult)
            wv = pc.tile([H, W], BF16, tag=f"wv{nm}")
            nc.vector.tensor_tensor(wv[:], t_w[:], t_m1[:], ALU.mult)
            wviews[nm] = wv
        for wi, nm in enumerate(("wx0", "wx1", "wy0", "wy1")):
            nc.sync.dma_start(d_wsc[wi:wi + 1, :], wviews[nm][:])
        t_idx = hwt("idxf")
        nc.vector.tensor_scalar(t_m1[:], t_y0[:], -1.0, float(H - 1),
                                ALU.max, ALU.min)
        nc.vector.tensor_scalar(t_m2[:], t_x0[:], -1.0, float(W - 1),
                                ALU.max, ALU.min)
        nc.vector.tensor_scalar(t_idx[:], t_m1[:], float(W), float(PAD),
                                ALU.mult, ALU.add)
        nc.vector.tensor_tensor(t_idx[:], t_idx[:], t_m2[:], ALU.add)

        # ---- interleave idx -> il [96, 960] int16 ----
        t_il = pc.tile([128, 960], I16)
        with tc.tile_pool(name="ps_tr", bufs=2,
                          space=bass.MemorySpace.PSUM) as ps_tr:
            for u in range(10):
                p_t = ps_tr.tile([16, 96], F32, tag="pt", name="p_t")
                nc.tensor.transpose(p_t[:], t_idx[:, 16 * u:16 * u + 16],
                                    t_id96[:])
                nc.vector.tensor_copy(t_il[0:16, u::10], p_t[:])
        k = 16
        while k < 128:
            n = min(k, 128 - k)
            nc.sync.dma_start(t_il[k:k + n, :], t_il[0:n, :])
            k += n
        if debug:
            nc.sync.dma_start(dbg["il"][:], t_il[:])
        pp_cm.__exit__(None, None, None)

        pcv = top.enter_context(tc.tile_pool(name="pcv", bufs=1))
        t_corr2 = pcv.tile([114, CPH, CPW], BF16)
        # prefetch f1 (needed only at corr) so the DMA hides under warp;
        # own pool (closed after corr) so conv4's tiles fit
        pf1_cm = tc.tile_pool(name="pf1", bufs=1)
        pf1 = pf1_cm.__enter__()
        t_f1b = pf1.tile([96, HW], BF16)
        nc.sync.dma_start(t_f1b[:], d_f1b[:])

        # ---- warp phase ----
        pmA_cm = tc.tile_pool(name="pmA", bufs=1)
        pmA = pmA_cm.__enter__()
        t_warp = pmA.tile([96, WPH, WPW], BF16)
        nc.vector.memset(t_warp[:, 0:3, :], 0.0)
        nc.vector.memset(t_warp[:, WPH - 3:WPH, :], 0.0)
        nc.vector.memset(t_warp[:, 3:WPH - 3, 0:3], 0.0)
        nc.vector.memset(t_warp[:, 3:WPH - 3, WPW - 3:WPW], 0.0)

        with tc.tile_pool(name="pw", bufs=1) as pw, \
                tc.tile_pool(name="pg", bufs=1) as pg:
            for q in range(4):
                r0 = q * 24
                repx0 = pw.tile([96, QP], BF16, tag="repA", name="repx0")
                repx1 = pw.tile([96, QP], BF16, tag="repB", name="repx1")
                for rep, wi in ((repx0, 0), (repx1, 1)):
                    src = d_wsc[wi:wi + 1, q * QP:(q + 1) * QP]
                    rsrc = bass.AP(src.tensor, src.offset,
                                   [[0, 96]] + list(src.ap)[1:])
                    nc.sync.dma_start(rep[:], rsrc)
                t_r0 = pw.tile([96, QP], BF16, tag="R0")
                t_r1 = pw.tile([96, QP], BF16, tag="R1")
                g00 = pg.tile([128, 1, QP], BF16, tag="g00")
                g01 = pg.tile([128, 1, QP], BF16, tag="g01")
                g10 = pg.tile([128, 1, QP], BF16, tag="g10")
                g11 = pg.tile([128, 1, QP], BF16, tag="g11")
                # c5-major emission keeps queue_num == (program-order index
                # among SWDGE DMAs) % 4, so each tile DMASW lane (8, round-
                # robin) sees updates from a single queue (interp invariant)
                taps = ((g00, 0), (g01, 1), (g10, W), (g11, W + 1))
                for c5 in range(5):
                    for ti, (gt, off) in enumerate(taps):
                        nc.gpsimd.dma_gather(
                            gt[:, :, 768 * c5:768 * (c5 + 1)],
                            d_f2t[off:off + NEG, :],
                            t_il[:, 240 * q + 48 * c5:240 * q + 48 * (c5 + 1)],
                            num_idxs=768, num_idxs_reg=768, elem_size=128,
                            transpose=True, queue_num=ti)
                ta = pg.tile([96, QP], BF16, tag="ta")
                tb = pg.tile([96, QP], BF16, tag="tb")
                nc.vector.tensor_tensor(ta[:], g00[0:96, 0, :], repx0[:],
                                        ALU.mult)
                nc.vector.tensor_tensor(tb[:], g01[0:96, 0, :], repx1[:],
                                        ALU.mult)
                nc.vector.tensor_tensor(t_r0[:], ta[:], tb[:], ALU.add)
                nc.vector.tensor_tensor(ta[:], g10[0:96, 0, :], repx0[:],
                                        ALU.mult)
                nc.vector.tensor_tensor(tb[:], g11[0:96, 0, :], repx1[:],
                                        ALU.mult)
                nc.vector.tensor_tensor(t_r1[:], ta[:], tb[:], ALU.add)
                repy0 = pw.tile([96, QP], BF16, tag="repA", name="repy0")
                repy1 = pw.tile([96, QP], BF16, tag="repB", name="repy1")
                for rep, wi in ((repy0, 2), (repy1, 3)):
                    src = d_wsc[wi:wi + 1, q * QP:(q + 1) * QP]
                    rsrc = bass.AP(src.tensor, src.offset,
                                   [[0, 96]] + list(src.ap)[1:])
                    nc.sync.dma_start(rep[:], rsrc)
                nc.vector.tensor_tensor(t_r0[:], t_r0[:], repy0[:], ALU.mult)
                nc.vector.tensor_tensor(t_r1[:], t_r1[:], repy1[:], ALU.mult)
                wdst = t_warp[:, 3 + r0:3 + r0 + 24, 3:3 + W]
                nc.vector.tensor_tensor(
                    wdst, t_r0[:].rearrange("p (r w) -> p r w", w=W),
                    t_r1[:].rearrange("p (r w) -> p r w", w=W), ALU.add)

        if debug:
            nc.sync.dma_start(dbg["warped"][:], t_warp[:])
            nc.sync.dma_start(dbg["flow"][0], t_flx[:])
            nc.sync.dma_start(dbg["flow"][1], t_fly[:])

        # ---- correlation ----
        nc.vector.memset(t_corr2[32:64], 0.0)
        nc.vector.memset(t_corr2[:, 0:1, :], 0.0)
        nc.vector.memset(t_corr2[:, CPH - 1:CPH, :], 0.0)
        nc.vector.memset(t_corr2[:, 1:CPH - 1, 0:1], 0.0)
        nc.vector.memset(t_corr2[:, 1:CPH - 1, W + 1:CPW], 0.0)

        with tc.tile_pool(name="pcr", bufs=4) as pcr, \
                tc.tile_pool(name="pst", bufs=4) as pst, \
                tc.tile_pool(name="ps_c", bufs=4,
                             space=bass.MemorySpace.PSUM) as ps_c, \
                tc.tile_pool(name="ps_p", bufs=2,
                             space=bass.MemorySpace.PSUM) as ps_p:
            for band in range(6):
                Y = band * 16
                for grp in range(3):
                    npair = 4 if grp < 2 else 2
                    p_pa = ps_p.tile([50, 512], BF16, tag="packa")
                    p_pb = ps_p.tile([50, 512], BF16, tag="packb")
                    for pj in range(npair):
                        stk = pst.tile([128, 100], BF16, tag="stk")
                        for half in range(2):
                            tx = grp * 8 + pj * 2 + half
                            X = tx * 8
                            p_c = ps_c.tile([128, 308], F32, tag="pcorr")
                            ti128 = (band * 20 + tx) * 128
                            nc.tensor.matmul(
                                p_c[:], t_f1b[:, ti128:ti128 + 128],
                                t_warp[:, Y:Y + 22, X:X + 14],
                                start=True, stop=True)
                            sb = pcr.tile([128, 308], BF16, tag="sbc")
                            if half == 0:
                                nc.vector.tensor_copy(sb[:], p_c[:])
                            else:
                                nc.scalar.activation(sb[:], p_c[:], ACTF.Copy)
                            nc.gpsimd.local_scatter(
                                stk[:, half * 50:half * 50 + 50], sb[:],
                                t_lsi[:], channels=128, num_elems=50,
                                num_idxs=308)
                        nc.tensor.transpose(
                            p_pa[:, pj * 128:(pj + 1) * 128], stk[:, 0:50],
                            t_id128b[:])
                        nc.tensor.transpose(
                            p_pb[:, pj * 128:(pj + 1) * 128], stk[:, 50:100],
                            t_id128b[:])
                    for half in range(2):
                        xbase = grp * 64 + half * 8
                        cv = t_corr2[:]
                        shp = [[CPH * CPW, 50], [16, npair], [CPW, 16], [1, 8]]
                        dst = bass.AP(
                            cv.tensor,
                            cv.offset + (1 + Y) * CPW + (1 + xbase), shp)
                        # kx1-tap duplicate: rows 64:114, cols shifted by -1
                        dst2 = bass.AP(
                            cv.tensor,
                            cv.offset + 64 * (CPH * CPW) + (1 + Y) * CPW
                            + xbase, shp)
                        src = (p_pa if half == 0 else p_pb)[:]
                        src = src.rearrange("p (j r c) -> p j r c", r=16, c=8)
                        src = src[:, 0:npair]
                        # fused lrelu on copy-out (ACT), then partition-
                        # shifted duplicate for the conv1 kx1 K-pack (DVE)
                        nc.scalar.activation(dst, src, ACTF.Prelu,
                                             bias=0.0, scale=1.0,
                                             alpha=t_al128[0:50])
                        nc.vector.tensor_copy(dst2, dst)

        pmA_cm.__exit__(None, None, None)
        pf1_cm.__exit__(None, None, None)

        if debug:
            nc.sync.dma_start(dbg["corr2"][:], t_corr2[:])

        # ---- convs ----
        pcv2 = top.enter_context(tc.tile_pool(name="pcv2", bufs=1))
        t_h1 = pcv2.tile([128, CPH, CPW], BF16)
        nc.vector.memset(t_h1[:, 0:1, :], 0.0)
        nc.vector.memset(t_h1[:, CPH - 1:CPH, :], 0.0)
        nc.vector.memset(t_h1[:, 1:CPH - 1, 0:1], 0.0)
        nc.vector.memset(t_h1[:, 1:CPH - 1, W + 1:CPW], 0.0)

        with tc.tile_pool(name="ps_cv", bufs=4,
                          space=bass.MemorySpace.PSUM) as ps_cv:
            # conv1
            for ch in range(NCH):
                r = 3 * ch
                p_o = ps_cv.tile([128, CHP], F32, tag="cvo")
                for ky in range(3):
                    nc.tensor.matmul(
                        p_o[:], t_c1p[:, ky * 128:(ky + 1) * 128],
                        t_corr2[0:114, r + ky:r + ky + 3, 0:W],
                        start=(ky == 0), stop=False)
                    nc.tensor.matmul(
                        p_o[:], t_c1s[:, ky * 128:(ky + 1) * 128],
                        t_corr2[0:50, r + ky:r + ky + 3, 2:2 + W],
                        start=False, stop=(ky == 2))
                nc.scalar.activation(
                    t_h1[:, r + 1:r + 4, 1:1 + W],
                    p_o[:].rearrange("p (r w) -> p r w", w=W),
                    ACTF.Prelu, bias=t_b1[:], scale=1.0, alpha=t_al128[:])
            if debug:
                nc.sync.dma_start(dbg["h1"][:], t_h1[:])

            # conv2
            t_h2 = pcv2.tile([128, CPH, CPW], BF16)
            nc.vector.memset(t_h2[:, 0:1, :], 0.0)
            nc.vector.memset(t_h2[:, CPH - 1:CPH, :], 0.0)
            nc.vector.memset(t_h2[:, 1:CPH - 1, 0:1], 0.0)
            nc.vector.memset(t_h2[:, 1:CPH - 1, W + 1:CPW], 0.0)
            for ch in range(NCH):
                r = 3 * ch
                p_o = ps_cv.tile([64, CHP], F32, tag="cvo")
                ti = 0
                for ky in range(3):
                    for kx in range(3):
                        nc.tensor.matmul(
                            p_o[:], t_c2[:, ti * 64:(ti + 1) * 64],
                            t_h1[:, r + ky:r + ky + 3, kx:kx + W],
                            start=(ti == 0), stop=(ti == 8))
                        ti += 1
                nc.scalar.activation(
                    t_h2[0:64, r + 1:r + 4, 1:1 + W],
                    p_o[:].rearrange("p (r w) -> p r w", w=W),
                    ACTF.Prelu, bias=t_b2[:], scale=1.0, alpha=t_al64[:])
                # kx1-tap duplicate for conv3 K-pack: partitions 64:128,
                # cols shifted by -1 (DVE, overlaps next chunk's matmuls)
                nc.vector.tensor_copy(
                    t_h2[64:128, r + 1:r + 4, 0:W],
                    t_h2[0:64, r + 1:r + 4, 1:1 + W])

            # conv3 -> h3 (padded 100x165 @ (2,2)); the 3 row-shifted
            # ky planes for conv4's K-pack are built chunk-by-chunk with
            # partition-shifted engine copies that overlap the matmuls
            t_h3 = pcv2.tile([128, QPH, QPW], BF16)
            nc.vector.memset(t_h3[0:32, 0:2, :], 0.0)
            nc.vector.memset(t_h3[0:32, QPH - 2:QPH, :], 0.0)
            nc.vector.memset(t_h3[:, 2:QPH - 2, 0:2], 0.0)
            nc.vector.memset(t_h3[:, 2:QPH - 2, W + 2:QPW], 0.0)
            nc.vector.memset(t_h3[32:64, 0:2, :], 0.0)
            nc.vector.memset(t_h3[64:128, 0:2, :], 0.0)
            nc.vector.memset(t_h3[96:128, 95:96, :], 0.0)
            for ch in range(NCH):
                r = 3 * ch
                p_o = ps_cv.tile([32, CHP], F32, tag="cvo")
                for ky in range(3):
                    nc.tensor.matmul(
                        p_o[:], t_c3p[:, ky * 32:(ky + 1) * 32],
                        t_h2[0:128, r + ky:r + ky + 3, 0:W],
                        start=(ky == 0), stop=False)
                    nc.tensor.matmul(
                        p_o[:], t_c3s[:, ky * 32:(ky + 1) * 32],
                        t_h2[0:64, r + ky:r + ky + 3, 2:2 + W],
                        start=False, stop=(ky == 2))
                nc.scalar.activation(
                    t_h3[0:32, r + 2:r + 5, 2:2 + W],
                    p_o[:].rearrange("p (r w) -> p r w", w=W),
                    ACTF.Prelu, bias=t_b3[:], scale=1.0, alpha=t_al32[:])
                for dr, eng in ((1, nc.vector), (2, nc.gpsimd),
                                (3, nc.vector)):
                    lo = max(0, r + 2 - dr)
                    hi = r + 5 - dr
                    eng.tensor_copy(
                        t_h3[32 * dr:32 * dr + 32, lo:hi, 2:2 + W],
                        t_h3[0:32, lo + dr:hi + dr, 2:2 + W])
            if debug:
                nc.sync.dma_start(dbg["h3"][:], t_h3[:])

            # conv4 + final add (flow flattened per quarter) + store
            with tc.tile_pool(name="po4", bufs=3) as po4:
                for qg in range(4):
                    t_flfq = po4.tile([2, QP], F32, tag="flfq", bufs=2)
                    rq = qg * 24
                    nc.sync.dma_start(t_flfq[0:1, :], t_flx[rq:rq + 24, :])
                    nc.sync.dma_start(t_flfq[1:2, :], t_fly[rq:rq + 24, :])
                    nc.vector.tensor_scalar(t_flfq[:], t_flfq[:], t_b4[:],
                                            None, ALU.add)
                    t_oq = po4.tile([2, QP], F32, tag="oq", bufs=2)
                    for cc in range(8):
                        ch = qg * 8 + cc
                        r = 3 * ch
                        p_o = ps_cv.tile([2, CHP], F32, tag="cvo")
                        for kx in range(5):
                            nc.tensor.matmul(
                                p_o[:], t_c4q[:, kx * 2:kx * 2 + 2],
                                t_h3[0:128, r:r + 3, kx:kx + W],
                                start=(kx == 0), stop=False)
                            nc.tensor.matmul(
                                p_o[:], t_c4s[:, kx * 2:kx * 2 + 2],
                                t_h3[0:32, r + 4:r + 7, kx:kx + W],
                                start=False, stop=(kx == 4))
                        nc.vector.tensor_tensor(
                            t_oq[:, cc * CHP:(cc + 1) * CHP], p_o[:],
                            t_flfq[:, cc * CHP:(cc + 1) * CHP], ALU.add)
                    nc.sync.dma_start(
                        d_out[:, rq:rq + 24, :],
                        t_oq[:].rearrange("p (r w) -> p r w", w=W))

    nc.compile()
    return nc


_STATE = {}


def _make_runner(nc):
    """Build a persistent jitted shard_map callable for the compiled Bass
    module (mirrors bass2jax.run_bass_via_pjrt, but reusable + exposes
    device placement for steady-state timing)."""
    import jax
    import numpy as _np
    from jax.sharding import Mesh, PartitionSpec, NamedSharding
    from jax.experimental.shard_map import shard_map
    from concourse import bass2jax as b2j
    from concourse import mybir as _mb

    b2j.install_neuronx_cc_hook()
    partition_name = (nc.partition_id_tensor.name
                      if nc.partition_id_tensor else None)
    in_names, out_names, out_avals, zero_outs = [], [], [], []
    for alloc in nc.m.functions[0].allocations:
        if not isinstance(alloc, _mb.MemoryLocationSet):
            continue
        name = alloc.memorylocations[0].name
        if alloc.kind == "ExternalInput":
            if name != partition_name:
                in_names.append(name)
        elif alloc.kind == "ExternalOutput":
            shape = tuple(alloc.tensor_shape)
            dtype = _mb.dt.np(alloc.dtype)
            out_names.append(name)
            out_avals.append(jax.core.ShapedArray(shape, dtype))
            zero_outs.append(_np.zeros(shape, dtype))
    n_params = len(in_names)
    all_in = list(in_names) + list(out_names)
    if partition_name is not None:
        all_in.append(partition_name)

    def _body(*args):
        operands = list(args)
        if partition_name is not None:
            operands.append(b2j.partition_id_tensor())
        outs = b2j._bass_exec_p.bind(
            *operands,
            out_avals=tuple(out_avals),
            in_names=tuple(all_in),
            out_names=tuple(out_names),
            lowering_input_output_aliases=(),
            sim_require_finite=True,
            sim_require_nnan=True,
            nc=nc,
        )
        return tuple(outs)

    devices = jax.devices()[:N_CORES]
    mesh = Mesh(np.asarray(devices), ("core",))
    nsh = len(in_names) + len(out_names)
    sharded = jax.jit(
        shard_map(_body, mesh=mesh,
                  in_specs=(PartitionSpec("core"),) * nsh,
                  out_specs=(PartitionSpec("core"),) * len(out_names),
                  check_rep=False),
        keep_unused=True)
    sharding = NamedSharding(mesh, PartitionSpec("core"))
    return {
        "in_names": in_names, "out_names": out_names,
        "zero_outs": zero_outs, "sharded": sharded, "sharding": sharding,
        "out_avals": out_avals,
    }


def _get_state(debug=False):
    key = "dbg" if debug else "main"
    if key not in _STATE:
        nc = build_program(debug=debug)
        _STATE[key] = {"nc": nc, "consts": _host_consts(),
                       "runner": _make_runner(nc)}
    return _STATE[key]


def _build_in_maps(feat_one, feat_two, flow_prev, up_w,
                   w1, b1, w2, b2, w3, b3, w4, b4, consts):
    ws = _host_weights(np.asarray(up_w, np.float32),
                       np.asarray(w1, np.float32), np.asarray(b1, np.float32),
                       np.asarray(w2, np.float32), np.asarray(b2, np.float32),
                       np.asarray(w3, np.float32), np.asarray(b3, np.float32),
                       np.asarray(w4, np.float32), np.asarray(b4, np.float32))
    shared = {"xg": consts["xg"], "yg": consts["yg"], "id96": consts["id96"],
              "id128b": consts["id128b"], "lsidx": consts["lsidx"]}
    for nm in ("upwtab", "c1p", "c1s", "c2", "c3p", "c3s", "c4q", "c4s",
               "b1", "b2", "b3", "b4", "al128", "al64", "al32"):
        shared[nm] = ws[nm]
    f1 = np.asarray(feat_one, np.float32).reshape(B, 96, HW)
    f2 = np.asarray(feat_two, np.float32).reshape(B, 96, HW)
    fp = np.asarray(flow_prev, np.float32)
    in_maps = []
    for i in range(N_CORES):
        m = dict(shared)
        f1t = (f1[i] * (1.0 / 96.0)).reshape(96, 6, 16, 20, 8)
        m["f1b"] = np.ascontiguousarray(
            f1t.transpose(0, 1, 3, 2, 4)).reshape(96, HW).astype(bf)
        ft = np.zeros((F2LEN, 128), bf)
        ft[PAD:PAD + HW, 0:96] = f2[i].T
        m["f2t"] = ft
        m["fp"] = fp[i]
        in_maps.append(m)
    return in_maps


def stage_inputs(in_maps, runner):
    """Concatenate per-core inputs on axis 0 and place on the 8 cores."""
    import jax
    args = []
    for nm in runner["in_names"]:
        args.append(np.concatenate([np.asarray(m[nm]) for m in in_maps],
                                   axis=0))
    for z in runner["zero_outs"]:
        args.append(np.zeros((N_CORES * z.shape[0], *z.shape[1:]), z.dtype))
    return [jax.device_put(a, runner["sharding"]) for a in args]


def run_staged(runner, dev_args):
    return runner["sharded"](*dev_args)


def kernel(feat_one, feat_two, flow_prev, up_w,
           w1, b1, w2, b2, w3, b3, w4, b4, debug=False):
    st = _get_state(debug)
    runner = st["runner"]
    in_maps = _build_in_maps(feat_one, feat_two, flow_prev, up_w,
                             w1, b1, w2, b2, w3, b3, w4, b4, st["consts"])
    dev_args = stage_inputs(in_maps, runner)
    outs = run_staged(runner, dev_args)
    oi = runner["out_names"].index("out")
    out = np.asarray(outs[oi]).reshape(N_CORES, 2, H, W).astype(np.float32)
    if debug:
        results = []
        for i in range(N_CORES):
            r = {}
            for j, nm in enumerate(runner["out_names"]):
                a = runner["out_avals"][j]
                r[nm] = np.asarray(outs[j]).reshape(N_CORES, *a.shape)[i]
            results.append(r)
        return out, results
    return out



# revision 38
# speedup vs baseline: 1.1330x; 1.1330x over previous
"""Trainium2 Bass kernel for nn_BasicVSR_LFN (upflow + backwarp + 7x7
correlation + 4 convs), data-parallel over batch: 1 sample per NeuronCore.

Per-core pipeline (shapes hardcoded for B=8, C=96, H=96, W=160):
  1. upflow (ConvTranspose2d 2->2, k4 s2 p1 groups=2) as 4 parity-plane
     stencils on DVE, assembled to flow[g][96,160] by strided DMA.
  2. warp coordinate/index/weight pipeline in [96y,160x] layout on DVE.
  3. idx -> interleaved int16 [96,960] (PE transpose + strided copies +
     partition-doubling DMA) for gpsimd ap_gather.
  4. backwarp: 4 ap_gather taps from padded f2 (AP offsets 0/1/160/161),
     bilinear combine on DVE; weights replicated across channel partitions
     by log-doubling DMAs per quarter. Output: padded channel-major warped
     bf16 [96, 102, 166].
  5. correlation: 120 PE matmuls (f1 16x8-pixel tile [96,128] x warped
     window [96,22,14]); PSUM [128,308] -> bf16 -> gpsimd local_scatter
     (static band table) -> [128,50] pixel-major -> PE pair transpose ->
     channel-major corr2 [100, 98, 163] (2x col-shift K-stack), lrelu.
  6. convs as K-packed shifted matmuls (N=480 row-aligned chunks), PSUM
     accumulation, fused bias+leaky-relu+bf16 cast via ACT Prelu copy-out.
     conv4 5x5 via 4-row-shift K-stack; out = flow + res.
"""
import numpy as np
import ml_dtypes

import concourse.bass as bass
import concourse.bacc as bacc
import concourse.mybir as mybir
import concourse.tile as tile
from contextlib import ExitStack

F32 = mybir.dt.float32
F32R = mybir.dt.float32r
BF16 = mybir.dt.float16  # fp16 everywhere (precision margin)
I16 = mybir.dt.int16
I32 = mybir.dt.int32
ALU = mybir.AluOpType
ACTF = mybir.ActivationFunctionType

B, C, H, W = 8, 96, 96, 160
HW = H * W
N_CORES = 8
PAD = 161
F2LEN = PAD + HW + PAD         # 15682
NEG = HW + PAD                 # 15521
AX = 2.5 * W / (W - 1.0)
AY = 2.5 * H / (H - 1.0)
EPSF = -0.5 + 2.0 ** -11

WPH, WPW = H + 6, W + 6        # 102, 166
CPH, CPW = H + 2, W + 3        # 98, 163
QPH, QPW = H + 4, W + 5        # 100, 165

GCH = 960
QP = 3840
NCH = 32                       # conv chunks (3 rows x 160)
CHP = 480

bf = np.float16

# upflow tap order (must match host table)
UP_ORDER = [(g, ry, rx, di, dj)
            for g in range(2) for ry in range(2) for rx in range(2)
            for di in ([-1, 0] if ry == 0 else [0, 1])
            for dj in ([-1, 0] if rx == 0 else [0, 1])]


def _host_consts():
    cs = {}
    cs["xg"] = np.tile(np.arange(W, dtype=np.float32)[None, :], (H, 1))
    cs["yg"] = np.tile(np.arange(H, dtype=np.float32)[:, None], (1, W))
    cs["id96"] = np.eye(96, dtype=np.float32)
    cs["id128b"] = np.eye(128, dtype=np.float32).astype(bf)
    lsi = np.full((128, 308), -1, dtype=np.int16)
    for m in range(128):
        r, c = m // 8, m % 8
        for n in range(308):
            Rr, Cc = n // 14, n % 14
            dy, dx = Rr - 3 - r, Cc - 3 - c
            if -3 <= dy <= 3 and -3 <= dx <= 3:
                lsi[m, n] = (dy + 3) * 7 + (dx + 3)
    cs["lsidx"] = lsi
    return cs


def _host_weights(up_w, w1, b1, w2, b2, w3, b3, w4, b4):
    ws = {}
    tab = np.zeros((48, 32), np.float32)
    for j, (g, ry, rx, di, dj) in enumerate(UP_ORDER):
        tab[:, j] = up_w[g, 0, 1 - 2 * di + ry, 1 - 2 * dj + rx]
    ws["upwtab"] = tab
    # conv1 pair [114, 3*128] (kx0 @ rows 0-48, kx1 @ rows 64-112),
    # single [50, 3*128]
    c1p = np.zeros((3, 114, 128), np.float32)
    c1s = np.zeros((3, 50, 128), np.float32)
    for ky in range(3):
        c1p[ky, 0:49] = w1[:, :, ky, 0].T
        c1p[ky, 64:113] = w1[:, :, ky, 1].T
        c1s[ky, 0:49] = w1[:, :, ky, 2].T
    ws["c1p"] = np.transpose(c1p, (1, 0, 2)).reshape(114, 384).astype(bf)
    ws["c1s"] = np.transpose(c1s, (1, 0, 2)).reshape(50, 384).astype(bf)
    c2 = np.zeros((9, 128, 64), np.float32)
    for ky in range(3):
        for kx in range(3):
            c2[ky * 3 + kx] = w2[:, :, ky, kx].T
    ws["c2"] = np.transpose(c2, (1, 0, 2)).reshape(128, 576).astype(bf)
    c3p = np.zeros((3, 128, 32), np.float32)
    c3s = np.zeros((3, 64, 32), np.float32)
    for ky in range(3):
        c3p[ky, 0:64] = w3[:, :, ky, 0].T
        c3p[ky, 64:128] = w3[:, :, ky, 1].T
        c3s[ky] = w3[:, :, ky, 2].T
    ws["c3p"] = np.transpose(c3p, (1, 0, 2)).reshape(128, 96).astype(bf)
    ws["c3s"] = np.transpose(c3s, (1, 0, 2)).reshape(64, 96).astype(bf)
    c4q = np.zeros((5, 128, 2), np.float32)
    c4s = np.zeros((5, 32, 2), np.float32)
    for kx in range(5):
        for dr in range(4):
            c4q[kx, dr * 32:(dr + 1) * 32] = w4[:, :, dr, kx].T
        c4s[kx] = w4[:, :, 4, kx].T
    ws["c4q"] = np.transpose(c4q, (1, 0, 2)).reshape(128, 10).astype(bf)
    ws["c4s"] = np.transpose(c4s, (1, 0, 2)).reshape(32, 10).astype(bf)
    # conv4 flow/bias injection matmul: out += I2 @ [flow; 1] rows + b4
    c4f = np.zeros((3, 2), np.float32)
    c4f[0, 0] = 1.0
    c4f[1, 1] = 1.0
    c4f[2, :] = np.asarray(b4, np.float32)
    ws["c4f"] = c4f
    for nm, b_ in (("b1", b1), ("b2", b2), ("b3", b3)):
        ws[nm] = np.asarray(b_, np.float32)[:, None]
    ws["al128"] = np.full((128, 1), 0.1, np.float32)
    ws["al64"] = np.full((64, 1), 0.1, np.float32)
    ws["al32"] = np.full((32, 1), 0.1, np.float32)
    return ws


def build_program(debug=False, sim_q0=False):
    # sim_q0: force all SWDGE gathers onto queue 0 — satisfies the
    # interp's lane/queue lock for local profiling; HW builds keep the
    # 4-queue assignment
    nc = bacc.Bacc("TRN2", target_bir_lowering=False, debug=False,
                   num_devices=N_CORES, num_swdge_queues=4)
    P = nc.declare_dram_parameter
    d_f1b = P("f1b", [96, HW], BF16, isOutput=False)
    d_f2t = P("f2t", [F2LEN, 128], BF16, isOutput=False)
    d_fp = P("fp", [2, 48, 80], F32, isOutput=False)
    d_upw = P("upwtab", [48, 32], F32, isOutput=False)
    d_xg = P("xg", [H, W], F32, isOutput=False)
    d_yg = P("yg", [H, W], F32, isOutput=False)
    d_id96 = P("id96", [96, 96], F32, isOutput=False)
    d_id128b = P("id128b", [128, 128], BF16, isOutput=False)
    d_lsi = P("lsidx", [128, 308], I16, isOutput=False)
    d_c1p = P("c1p", [114, 384], BF16, isOutput=False)
    d_c1s = P("c1s", [50, 384], BF16, isOutput=False)
    d_c2 = P("c2", [128, 576], BF16, isOutput=False)
    d_c3p = P("c3p", [128, 96], BF16, isOutput=False)
    d_c3s = P("c3s", [64, 96], BF16, isOutput=False)
    d_c4q = P("c4q", [128, 10], BF16, isOutput=False)
    d_c4s = P("c4s", [32, 10], BF16, isOutput=False)
    d_c4f = P("c4f", [3, 2], F32, isOutput=False)
    d_b1 = P("b1", [128, 1], F32, isOutput=False)
    d_b2 = P("b2", [64, 1], F32, isOutput=False)
    d_b3 = P("b3", [32, 1], F32, isOutput=False)
    d_al128 = P("al128", [128, 1], F32, isOutput=False)
    d_al64 = P("al64", [64, 1], F32, isOutput=False)
    d_al32 = P("al32", [32, 1], F32, isOutput=False)
    d_out = P("out", [2, H, W], F32, isOutput=True)
    d_wsc = nc.dram_tensor("wscratch", [4, HW], BF16)
    dbg = {}
    if debug:
        dbg["warped"] = P("dbg_warped", [96, WPH, WPW], BF16, isOutput=True)
        dbg["corr2"] = P("dbg_corr2", [114, CPH, CPW], BF16, isOutput=True)
        dbg["h1"] = P("dbg_h1", [128, CPH, CPW], BF16, isOutput=True)
        dbg["h3"] = P("dbg_h3", [128, QPH, QPW], BF16, isOutput=True)
        dbg["flow"] = P("dbg_flow", [2, H, W], F32, isOutput=True)
        dbg["il"] = P("dbg_il", [128, 960], I16, isOutput=True)

    with tile.TileContext(nc) as tc, ExitStack() as top:
        pc = top.enter_context(tc.tile_pool(name="pc", bufs=1))

        # ---- consts ----
        t_upw = pc.tile([48, 32], F32)
        t_id96 = pc.tile([96, 96], F32)
        t_id128b = pc.tile([128, 128], BF16)
        t_lsi = pc.tile([128, 308], I16)
        t_c1p = pc.tile([114, 384], BF16)
        t_c1s = pc.tile([50, 384], BF16)
        t_c2 = pc.tile([128, 576], BF16)
        t_c3p = pc.tile([128, 96], BF16)
        t_c3s = pc.tile([64, 96], BF16)
        t_c4q = pc.tile([128, 10], BF16)
        t_c4s = pc.tile([32, 10], BF16)
        t_c4f = pc.tile([3, 2], F32)
        t_b1 = pc.tile([128, 1], F32)
        t_b2 = pc.tile([64, 1], F32)
        t_b3 = pc.tile([32, 1], F32)
        t_al128 = pc.tile([128, 1], F32)
        t_al64 = pc.tile([64, 1], F32)
        t_al32 = pc.tile([32, 1], F32)
        for tt, dd in ((t_upw, d_upw),
                       (t_id96, d_id96), (t_id128b, d_id128b),
                       (t_lsi, d_lsi), (t_c1p, d_c1p), (t_c1s, d_c1s),
                       (t_c2, d_c2), (t_c3p, d_c3p), (t_c3s, d_c3s),
                       (t_c4q, d_c4q), (t_c4s, d_c4s), (t_c4f, d_c4f),
                       (t_b1, d_b1),
                       (t_b2, d_b2), (t_b3, d_b3),
                       (t_al128, d_al128), (t_al64, d_al64),
                       (t_al32, d_al32)):
            nc.sync.dma_start(tt[:], dd[:])

        # ---- upflow ----
        pp_cm = tc.tile_pool(name="pp", bufs=1)
        pp = pp_cm.__enter__()
        t_xg = pp.tile([H, W], F32)
        t_yg = pp.tile([H, W], F32)
        nc.sync.dma_start(t_xg[:], d_xg[:])
        nc.sync.dma_start(t_yg[:], d_yg[:])
        # fps[g][di+1]: flow_prev[g, p+di, q+dj] readable at col offset dj+1
        fps = {}
        for g in range(2):
            for di in (-1, 0, 1):
                nm = f"fps{g}_{di + 1}"
                t = pp.tile([48, 82], F32, tag=nm, name=nm)
                nc.vector.memset(t[:], 0.0)
                lo, hi = max(0, di), min(48, 48 + di)
                nc.sync.dma_start(t[lo - di:hi - di, 1:81], d_fp[g, lo:hi, :])
                fps[(g, di)] = t
        t_upt = pp.tile([48, 80], F32, tag="uptmp")
        planes = {}
        for key in {(g, ry, rx) for (g, ry, rx, _, _) in UP_ORDER}:
            nm = f"pl{key[0]}{key[1]}{key[2]}"
            planes[key] = pp.tile([48, 80], F32, tag=nm, name=nm)
        done = set()
        for j, (g, ry, rx, di, dj) in enumerate(UP_ORDER):
            pl = planes[(g, ry, rx)]
            sc = t_upw[:, j:j + 1]
            src = fps[(g, di)][:, 1 + dj:81 + dj]
            if (g, ry, rx) not in done:
                done.add((g, ry, rx))
                nc.vector.tensor_scalar(pl[:], src, sc, None, ALU.mult)
            else:
                nc.vector.tensor_scalar(t_upt[:], src, sc, None, ALU.mult)
                nc.vector.tensor_tensor(pl[:], pl[:], t_upt[:], ALU.add)
        t_flx = pc.tile([H, W], F32)
        t_fly = pc.tile([H, W], F32)
        flyx = [t_flx, t_fly]
        for (g, ry, rx), pl in sorted(planes.items()):
            nc.sync.dma_start(flyx[g][ry::2, rx::2], pl[:])

        # ---- warp index / weight pipeline ----
        def hwt(tag, dt=F32):
            return pp.tile([H, W], dt, tag=tag, name=tag)

        t_px, t_py = hwt("px"), hwt("py")
        nc.vector.tensor_scalar(t_px[:], t_flx[:], AX, None, ALU.mult)
        nc.vector.tensor_tensor(t_px[:], t_px[:], t_xg[:], ALU.add)
        nc.vector.tensor_scalar(t_py[:], t_fly[:], AY, None, ALU.mult)
        nc.vector.tensor_tensor(t_py[:], t_py[:], t_yg[:], ALU.add)
        t_x0, t_y0 = hwt("x0"), hwt("y0")
        t_i32 = pp.tile([H, W], I32, tag="i32")
        nc.vector.tensor_scalar(t_x0[:], t_px[:], EPSF, None, ALU.add)
        nc.vector.tensor_copy(t_i32[:], t_x0[:])
        nc.vector.tensor_copy(t_x0[:], t_i32[:])
        nc.vector.tensor_scalar(t_y0[:], t_py[:], EPSF, None, ALU.add)
        nc.vector.tensor_copy(t_i32[:], t_y0[:])
        nc.vector.tensor_copy(t_y0[:], t_i32[:])
        t_wx1, t_wy1, t_wx0, t_wy0 = (hwt("wx1"), hwt("wy1"),
                                      hwt("wx0"), hwt("wy0"))
        nc.vector.tensor_tensor(t_wx1[:], t_px[:], t_x0[:], ALU.subtract)
        nc.vector.tensor_tensor(t_wy1[:], t_py[:], t_y0[:], ALU.subtract)
        nc.vector.tensor_scalar(t_wx0[:], t_wx1[:], -1.0, 1.0, ALU.mult, ALU.add)
        nc.vector.tensor_scalar(t_wy0[:], t_wy1[:], -1.0, 1.0, ALU.mult, ALU.add)
        t_m1, t_m2 = hwt("m1"), hwt("m2")
        wviews = {}
        for nm, t_base, t_w, lo, hi in (
                ("wx0", t_x0, t_wx0, 0.0, float(W - 1)),
                ("wx1", t_x0, t_wx1, -1.0, float(W - 2)),
                ("wy0", t_y0, t_wy0, 0.0, float(H - 1)),
                ("wy1", t_y0, t_wy1, -1.0, float(H - 2))):
            nc.vector.tensor_scalar(t_m1[:], t_base[:], lo, None, ALU.is_ge)
            nc.vector.tensor_scalar(t_m2[:], t_base[:], hi, None, ALU.is_le)
            nc.vector.tensor_tensor(t_m1[:], t_m1[:], t_m2[:], ALU.mult)
            wv = pc.tile([H, W], BF16, tag=f"wv{nm}")
            nc.vector.tensor_tensor(wv[:], t_w[:], t_m1[:], ALU.mult)
            wviews[nm] = wv
        # fuse the x/y weights into the 4 per-tap products w_ij =
        # wx_j * wy_i (kills the separate y-multiply stage in the warp
        # combine), then flatten each [H,W] plane to a d_wsc row
        wprod = {}
        for wi, (nm, nx, ny) in enumerate(
                (("w00", "wx0", "wy0"), ("w01", "wx1", "wy0"),
                 ("w10", "wx0", "wy1"), ("w11", "wx1", "wy1"))):
            wp = pc.tile([H, W], BF16, tag=f"wp{nm}")
            nc.vector.tensor_tensor(wp[:], wviews[nx][:], wviews[ny][:],
                                    ALU.mult)
            wprod[nm] = wp
            nc.sync.dma_start(d_wsc[wi:wi + 1, :], wp[:])
        t_idx = hwt("idxf")
        nc.vector.tensor_scalar(t_m1[:], t_y0[:], -1.0, float(H - 1),
                                ALU.max, ALU.min)
        nc.vector.tensor_scalar(t_m2[:], t_x0[:], -1.0, float(W - 1),
                                ALU.max, ALU.min)
        nc.vector.tensor_scalar(t_idx[:], t_m1[:], float(W), float(PAD),
                                ALU.mult, ALU.add)
        nc.vector.tensor_tensor(t_idx[:], t_idx[:], t_m2[:], ALU.add)

        # ---- interleave idx -> il [96, 960] int16 ----
        t_il = pc.tile([128, 960], I16)
        with tc.tile_pool(name="ps_tr", bufs=2,
                          space=bass.MemorySpace.PSUM) as ps_tr:
            for u in range(10):
                p_t = ps_tr.tile([16, 96], F32, tag="pt", name="p_t")
                nc.tensor.transpose(p_t[:], t_idx[:, 16 * u:16 * u + 16],
                                    t_id96[:])
                nc.vector.tensor_copy(t_il[0:16, u::10], p_t[:])
        k = 16
        while k < 128:
            n = min(k, 128 - k)
            nc.sync.dma_start(t_il[k:k + n, :], t_il[0:n, :])
            k += n
        if debug:
            nc.sync.dma_start(dbg["il"][:], t_il[:])
        pp_cm.__exit__(None, None, None)

        pcv = top.enter_context(tc.tile_pool(name="pcv", bufs=1))
        t_corr2 = pcv.tile([114, CPH, CPW], BF16)

        # ---- warp phase ----
        pmA_cm = tc.tile_pool(name="pmA", bufs=1)
        pmA = pmA_cm.__enter__()
        t_warp = pmA.tile([96, WPH, WPW], BF16)
        nc.vector.memset(t_warp[:, 0:3, :], 0.0)
        nc.vector.memset(t_warp[:, WPH - 3:WPH, :], 0.0)
        nc.vector.memset(t_warp[:, 3:WPH - 3, 0:3], 0.0)
        nc.vector.memset(t_warp[:, 3:WPH - 3, WPW - 3:WPW], 0.0)

        # per quarter: 2 paired gathers x 5 chunks (elem 256 @ step 128
        # fetches the x/x+1 taps together), 4 weight-product broadcasts,
        # in-place bilinear combine. bufs=2 pipelines quarters.
        with tc.tile_pool(name="pw", bufs=2) as pw, \
                tc.tile_pool(name="pg", bufs=2) as pg:
            for q in range(4):
                r0 = q * 24
                reps = {}
                for wi, nm in enumerate(("w00", "w01", "w10", "w11")):
                    rep = pw.tile([96, QP], BF16, tag=f"rp{nm}", name=nm)
                    src = d_wsc[wi:wi + 1, q * QP:(q + 1) * QP]
                    rsrc = bass.AP(src.tensor, src.offset,
                                   [[0, 96]] + list(src.ap)[1:])
                    nc.sync.dma_start(rep[:], rsrc)
                    reps[nm] = rep
                gA = pg.tile([128, 6, 2, 640], BF16, tag="gA")
                gB = pg.tile([128, 6, 2, 640], BF16, tag="gB")
                for c6 in range(6):
                    for ti, (gt, off) in enumerate(((gA, 0), (gB, W))):
                        src = bass.AP(d_f2t[:].tensor,
                                      d_f2t[:].offset + off * 128,
                                      [[128, NEG], [1, 256]])
                        nc.gpsimd.dma_gather(
                            gt[:, c6, :, :], src,
                            t_il[:, 240 * q + 40 * c6:240 * q + 40 * (c6 + 1)],
                            num_idxs=640, num_idxs_reg=640, elem_size=256,
                            elem_step=128, transpose=True,
                            # single queue for ALL gathers: lane sems are
                            # cumulative, so mixing queues on one lane lets
                            # a later gather satisfy an earlier wait
                            # threshold out of order -> data races. One
                            # queue = global FIFO = sound thresholds.
                            queue_num=0)
                # chunk = 640 px = exactly 4 warp rows, so the strided
                # [96, 6, 4, 160] views line up with the padded t_warp
                gA0 = gA[0:96, :, 0, :]
                gA1 = gA[0:96, :, 1, :]
                gB0 = gB[0:96, :, 0, :]
                gB1 = gB[0:96, :, 1, :]
                rw = {nm: reps[nm][:].rearrange("p (c n) -> p c n", c=6)
                      for nm in reps}
                nc.vector.tensor_tensor(gA0, gA0, rw["w00"], ALU.mult)
                nc.vector.tensor_tensor(gA1, gA1, rw["w01"], ALU.mult)
                nc.vector.tensor_tensor(gA0, gA0, gA1, ALU.add)
                nc.vector.tensor_tensor(gB0, gB0, rw["w10"], ALU.mult)
                nc.vector.tensor_tensor(gB1, gB1, rw["w11"], ALU.mult)
                nc.vector.tensor_tensor(gB0, gB0, gB1, ALU.add)
                wdst = t_warp[:, 3 + r0:3 + r0 + 24, 3:3 + W]
                wdst = wdst.rearrange("p (c r) w -> p c r w", r=4)
                nc.vector.tensor_tensor(
                    wdst, gA0.rearrange("p c (r w) -> p c r w", w=W),
                    gB0.rearrange("p c (r w) -> p c r w", w=W), ALU.add)
                # WAR guard: the strided final-add read above is missed by
                # the slot reuse tracking; these DVE writes are engine-
                # ordered after it, so next quarter's gathers (WAW) can't
                # overwrite gA/gB while it still reads them
                nc.vector.memset(gA[0:32, 0, 0, 0:2], 0.0)
                nc.vector.memset(gB[0:32, 0, 0, 0:2], 0.0)
                for nm, rep in reps.items():
                    nc.vector.memset(rep[0:32, 0:2], 0.0)

        if debug:
            nc.sync.dma_start(dbg["warped"][:], t_warp[:])
            nc.sync.dma_start(dbg["flow"][0], t_flx[:])
            nc.sync.dma_start(dbg["flow"][1], t_fly[:])

        # ---- correlation ----
        nc.vector.memset(t_corr2[32:64], 0.0)
        nc.vector.memset(t_corr2[:, 0:1, :], 0.0)
        nc.vector.memset(t_corr2[:, CPH - 1:CPH, :], 0.0)
        nc.vector.memset(t_corr2[:, 1:CPH - 1, 0:1], 0.0)
        nc.vector.memset(t_corr2[:, 1:CPH - 1, W + 1:CPW], 0.0)

        pf1_cm = tc.tile_pool(name="pf1", bufs=1)
        pf1 = pf1_cm.__enter__()
        t_f1b = pf1.tile([96, HW], BF16)
        # band-wise loads: band b's matmuls only wait on chunk b
        for b6 in range(6):
            nc.sync.dma_start(t_f1b[:, b6 * 2560:(b6 + 1) * 2560],
                              d_f1b[:, b6 * 2560:(b6 + 1) * 2560])

        with tc.tile_pool(name="pcr", bufs=4) as pcr, \
                tc.tile_pool(name="pst", bufs=4) as pst, \
                tc.tile_pool(name="ps_c", bufs=4,
                             space=bass.MemorySpace.PSUM) as ps_c, \
                tc.tile_pool(name="ps_p", bufs=2,
                             space=bass.MemorySpace.PSUM) as ps_p:
            for band in range(6):
                Y = band * 16
                for grp in range(3):
                    npair = 4 if grp < 2 else 2
                    p_pa = ps_p.tile([50, 512], BF16, tag="packa")
                    p_pb = ps_p.tile([50, 512], BF16, tag="packb")
                    for pj in range(npair):
                        stk = pst.tile([128, 100], BF16, tag="stk")
                        for half in range(2):
                            tx = grp * 8 + pj * 2 + half
                            X = tx * 8
                            p_c = ps_c.tile([128, 308], F32, tag="pcorr")
                            ti128 = (band * 20 + tx) * 128
                            nc.tensor.matmul(
                                p_c[:], t_f1b[:, ti128:ti128 + 128],
                                t_warp[:, Y:Y + 22, X:X + 14],
                                start=True, stop=True)
                            sb = pcr.tile([128, 308], BF16, tag="sbc")
                            if half == 0:
                                nc.vector.tensor_copy(sb[:], p_c[:])
                            else:
                                nc.scalar.activation(sb[:], p_c[:], ACTF.Copy)
                            nc.gpsimd.local_scatter(
                                stk[:, half * 50:half * 50 + 50], sb[:],
                                t_lsi[:], channels=128, num_elems=50,
                                num_idxs=308)
                        nc.tensor.transpose(
                            p_pa[:, pj * 128:(pj + 1) * 128], stk[:, 0:50],
                            t_id128b[:])
                        nc.tensor.transpose(
                            p_pb[:, pj * 128:(pj + 1) * 128], stk[:, 50:100],
                            t_id128b[:])
                    for half in range(2):
                        xbase = grp * 64 + half * 8
                        cv = t_corr2[:]
                        shp = [[CPH * CPW, 50], [16, npair], [CPW, 16], [1, 8]]
                        dst = bass.AP(
                            cv.tensor,
                            cv.offset + (1 + Y) * CPW + (1 + xbase), shp)
                        # kx1-tap duplicate: rows 64:114, cols shifted by -1
                        dst2 = bass.AP(
                            cv.tensor,
                            cv.offset + 64 * (CPH * CPW) + (1 + Y) * CPW
                            + xbase, shp)
                        src = (p_pa if half == 0 else p_pb)[:]
                        src = src.rearrange("p (j r c) -> p j r c", r=16, c=8)
                        src = src[:, 0:npair]
                        # fused lrelu on copy-out (ACT), then partition-
                        # shifted duplicate for the conv1 kx1 K-pack (DVE)
                        nc.scalar.activation(dst, src, ACTF.Prelu,
                                             bias=0.0, scale=1.0,
                                             alpha=t_al128[0:50])
                        nc.vector.tensor_copy(dst2, dst)

        pf1_cm.__exit__(None, None, None)
        pmA_cm.__exit__(None, None, None)

        if debug:
            nc.sync.dma_start(dbg["corr2"][:], t_corr2[:])

        # ---- convs ----
        pcv2 = top.enter_context(tc.tile_pool(name="pcv2", bufs=1))
        t_h1 = pcv2.tile([128, CPH, CPW], BF16)
        nc.vector.memset(t_h1[:, 0:1, :], 0.0)
        nc.vector.memset(t_h1[:, CPH - 1:CPH, :], 0.0)
        nc.vector.memset(t_h1[:, 1:CPH - 1, 0:1], 0.0)
        nc.vector.memset(t_h1[:, 1:CPH - 1, W + 1:CPW], 0.0)

        with tc.tile_pool(name="ps_cv", bufs=4,
                          space=bass.MemorySpace.PSUM) as ps_cv:
            # conv1
            for ch in range(NCH):
                r = 3 * ch
                p_o = ps_cv.tile([128, CHP], F32, tag="cvo")
                for ky in range(3):
                    nc.tensor.matmul(
                        p_o[:], t_c1p[:, ky * 128:(ky + 1) * 128],
                        t_corr2[0:114, r + ky:r + ky + 3, 0:W],
                        start=(ky == 0), stop=False)
                    nc.tensor.matmul(
                        p_o[:], t_c1s[:, ky * 128:(ky + 1) * 128],
                        t_corr2[0:50, r + ky:r + ky + 3, 2:2 + W],
                        start=False, stop=(ky == 2))
                nc.scalar.activation(
                    t_h1[:, r + 1:r + 4, 1:1 + W],
                    p_o[:].rearrange("p (r w) -> p r w", w=W),
                    ACTF.Prelu, bias=t_b1[:], scale=1.0, alpha=t_al128[:])
            if debug:
                nc.sync.dma_start(dbg["h1"][:], t_h1[:])

            # conv2 — chunk pairs run concurrently on the two 64-col PE
            # tiles (tile_position derives from the psum partition base)
            t_h2 = pcv2.tile([128, CPH, CPW], BF16)
            nc.vector.memset(t_h2[:, 0:1, :], 0.0)
            nc.vector.memset(t_h2[:, CPH - 1:CPH, :], 0.0)
            nc.vector.memset(t_h2[:, 1:CPH - 1, 0:1], 0.0)
            nc.vector.memset(t_h2[:, 1:CPH - 1, W + 1:CPW], 0.0)
            for pch in range(0, NCH, 2):
                p_o = ps_cv.tile([128, CHP], F32, tag="cvo")
                for ti in range(9):
                    ky, kx = divmod(ti, 3)
                    for half in range(2):
                        r = 3 * (pch + half)
                        nc.tensor.matmul(
                            p_o[64 * half:64 * half + 64],
                            t_c2[:, ti * 64:(ti + 1) * 64],
                            t_h1[:, r + ky:r + ky + 3, kx:kx + W],
                            start=(ti == 0), stop=(ti == 8),
                            skip_group_check=True,
                            tile_position=(0, 64 * half))
                for half in range(2):
                    r = 3 * (pch + half)
                    nc.scalar.activation(
                        t_h2[0:64, r + 1:r + 4, 1:1 + W],
                        p_o[64 * half:64 * half + 64].rearrange(
                            "p (r w) -> p r w", w=W),
                        ACTF.Prelu, bias=t_b2[:], scale=1.0, alpha=t_al64[:])
                    # kx1-tap duplicate for conv3 K-pack: partitions
                    # 64:128, cols shifted by -1 (DVE, overlaps matmuls)
                    nc.vector.tensor_copy(
                        t_h2[64:128, r + 1:r + 4, 0:W],
                        t_h2[0:64, r + 1:r + 4, 1:1 + W])

            # conv3 -> h3 (padded 100x165 @ (2,2)); the 3 row-shifted
            # ky planes for conv4's K-pack are built chunk-by-chunk with
            # partition-shifted engine copies that overlap the matmuls
            t_h3 = pcv2.tile([128, QPH, QPW], BF16)
            nc.vector.memset(t_h3[0:32, 0:2, :], 0.0)
            nc.vector.memset(t_h3[0:32, QPH - 2:QPH, :], 0.0)
            nc.vector.memset(t_h3[:, 2:QPH - 2, 0:2], 0.0)
            nc.vector.memset(t_h3[:, 2:QPH - 2, W + 2:QPW], 0.0)
            nc.vector.memset(t_h3[32:64, 0:2, :], 0.0)
            nc.vector.memset(t_h3[64:128, 0:2, :], 0.0)
            nc.vector.memset(t_h3[96:128, 95:96, :], 0.0)
            # chunk quads run concurrently on the four 32-col PE tiles
            for qch in range(0, NCH, 4):
                p_o = ps_cv.tile([128, CHP], F32, tag="cvo")
                for ky in range(3):
                    for m in range(4):
                        r = 3 * (qch + m)
                        nc.tensor.matmul(
                            p_o[32 * m:32 * m + 32],
                            t_c3p[:, ky * 32:(ky + 1) * 32],
                            t_h2[0:128, r + ky:r + ky + 3, 0:W],
                            start=(ky == 0), stop=False,
                            skip_group_check=True,
                            tile_position=(0, 32 * m))
                    for m in range(4):
                        r = 3 * (qch + m)
                        nc.tensor.matmul(
                            p_o[32 * m:32 * m + 32],
                            t_c3s[:, ky * 32:(ky + 1) * 32],
                            t_h2[0:64, r + ky:r + ky + 3, 2:2 + W],
                            start=False, stop=(ky == 2),
                            skip_group_check=True,
                            tile_position=(0, 32 * m))
                for m in range(4):
                    r = 3 * (qch + m)
                    nc.scalar.activation(
                        t_h3[0:32, r + 2:r + 5, 2:2 + W],
                        p_o[32 * m:32 * m + 32].rearrange(
                            "p (r w) -> p r w", w=W),
                        ACTF.Prelu, bias=t_b3[:], scale=1.0, alpha=t_al32[:])
                    for dr, eng in ((1, nc.vector), (2, nc.gpsimd),
                                    (3, nc.vector)):
                        lo = max(0, r + 2 - dr)
                        hi = r + 5 - dr
                        eng.tensor_copy(
                            t_h3[32 * dr:32 * dr + 32, lo:hi, 2:2 + W],
                            t_h3[0:32, lo + dr:hi + dr, 2:2 + W])
            if debug:
                nc.sync.dma_start(dbg["h3"][:], t_h3[:])

            # conv4: chunk quads on the four 32-col PE tiles; flow + bias
            # accumulated into PSUM via a K=3 identity/bias matmul (f32r)
            with tc.tile_pool(name="po4", bufs=3) as po4:
                for qg in range(4):
                    t_fl3 = po4.tile([3, QP], F32, tag="flfq", bufs=2)
                    rq = qg * 24
                    # engine memsets must start at partition 0/32/64/96:
                    # fill all 3 rows with 1.0, then overwrite 0:2 w/ flow
                    nc.vector.memset(t_fl3[0:3, :], 1.0)
                    nc.sync.dma_start(t_fl3[0:1, :], t_flx[rq:rq + 24, :])
                    nc.sync.dma_start(t_fl3[1:2, :], t_fly[rq:rq + 24, :])
                    t_oq = po4.tile([2, QP], F32, tag="oq", bufs=2)
                    for qc in range(0, 8, 4):
                        p_o = ps_cv.tile([128, CHP], F32, tag="cvo")
                        for kx in range(5):
                            for m in range(4):
                                r = 3 * (qg * 8 + qc + m)
                                nc.tensor.matmul(
                                    p_o[32 * m:32 * m + 2],
                                    t_c4q[:, kx * 2:kx * 2 + 2],
                                    t_h3[0:128, r:r + 3, kx:kx + W],
                                    start=(kx == 0), stop=False,
                                    skip_group_check=True,
                                    tile_position=(0, 32 * m))
                            for m in range(4):
                                r = 3 * (qg * 8 + qc + m)
                                nc.tensor.matmul(
                                    p_o[32 * m:32 * m + 2],
                                    t_c4s[:, kx * 2:kx * 2 + 2],
                                    t_h3[0:32, r + 4:r + 7, kx:kx + W],
                                    start=False, stop=False,
                                    skip_group_check=True,
                                    tile_position=(0, 32 * m))
                        for m in range(4):
                            cc = qc + m
                            nc.tensor.matmul(
                                p_o[32 * m:32 * m + 2],
                                t_c4f[:],
                                t_fl3[:, cc * CHP:(cc + 1) * CHP],
                                start=False, stop=True,
                                skip_group_check=True,
                                tile_position=(0, 32 * m))
                        for m in range(4):
                            cc = qc + m
                            nc.vector.tensor_copy(
                                t_oq[:, cc * CHP:(cc + 1) * CHP],
                                p_o[32 * m:32 * m + 2])
                    nc.sync.dma_start(
                        d_out[:, rq:rq + 24, :],
                        t_oq[:].rearrange("p (r w) -> p r w", w=W))

    nc.compile()
    return nc


_STATE = {}


def _make_runner(nc):
    """Build a persistent jitted shard_map callable for the compiled Bass
    module (mirrors bass2jax.run_bass_via_pjrt, but reusable + exposes
    device placement for steady-state timing)."""
    import jax
    import numpy as _np
    from jax.sharding import Mesh, PartitionSpec, NamedSharding
    from jax.experimental.shard_map import shard_map
    from concourse import bass2jax as b2j
    from concourse import mybir as _mb

    b2j.install_neuronx_cc_hook()
    partition_name = (nc.partition_id_tensor.name
                      if nc.partition_id_tensor else None)
    in_names, out_names, out_avals, zero_outs = [], [], [], []
    for alloc in nc.m.functions[0].allocations:
        if not isinstance(alloc, _mb.MemoryLocationSet):
            continue
        name = alloc.memorylocations[0].name
        if alloc.kind == "ExternalInput":
            if name != partition_name:
                in_names.append(name)
        elif alloc.kind == "ExternalOutput":
            shape = tuple(alloc.tensor_shape)
            dtype = _mb.dt.np(alloc.dtype)
            out_names.append(name)
            out_avals.append(jax.core.ShapedArray(shape, dtype))
            zero_outs.append(_np.zeros(shape, dtype))
    n_params = len(in_names)
    all_in = list(in_names) + list(out_names)
    if partition_name is not None:
        all_in.append(partition_name)

    def _body(*args):
        operands = list(args)
        if partition_name is not None:
            operands.append(b2j.partition_id_tensor())
        outs = b2j._bass_exec_p.bind(
            *operands,
            out_avals=tuple(out_avals),
            in_names=tuple(all_in),
            out_names=tuple(out_names),
            lowering_input_output_aliases=(),
            sim_require_finite=True,
            sim_require_nnan=True,
            nc=nc,
        )
        return tuple(outs)

    devices = jax.devices()[:N_CORES]
    mesh = Mesh(np.asarray(devices), ("core",))
    nsh = len(in_names) + len(out_names)
    sharded = jax.jit(
        shard_map(_body, mesh=mesh,
                  in_specs=(PartitionSpec("core"),) * nsh,
                  out_specs=(PartitionSpec("core"),) * len(out_names),
                  check_rep=False),
        keep_unused=True)
    sharding = NamedSharding(mesh, PartitionSpec("core"))
    return {
        "in_names": in_names, "out_names": out_names,
        "zero_outs": zero_outs, "sharded": sharded, "sharding": sharding,
        "out_avals": out_avals,
    }


def _get_state(debug=False):
    key = "dbg" if debug else "main"
    if key not in _STATE:
        nc = build_program(debug=debug)
        _STATE[key] = {"nc": nc, "consts": _host_consts(),
                       "runner": _make_runner(nc)}
    return _STATE[key]


def _build_in_maps(feat_one, feat_two, flow_prev, up_w,
                   w1, b1, w2, b2, w3, b3, w4, b4, consts):
    ws = _host_weights(np.asarray(up_w, np.float32),
                       np.asarray(w1, np.float32), np.asarray(b1, np.float32),
                       np.asarray(w2, np.float32), np.asarray(b2, np.float32),
                       np.asarray(w3, np.float32), np.asarray(b3, np.float32),
                       np.asarray(w4, np.float32), np.asarray(b4, np.float32))
    shared = {"xg": consts["xg"], "yg": consts["yg"], "id96": consts["id96"],
              "id128b": consts["id128b"], "lsidx": consts["lsidx"]}
    for nm in ("upwtab", "c1p", "c1s", "c2", "c3p", "c3s", "c4q", "c4s",
               "c4f", "b1", "b2", "b3", "al128", "al64", "al32"):
        shared[nm] = ws[nm]
    f1 = np.asarray(feat_one, np.float32).reshape(B, 96, HW)
    f2 = np.asarray(feat_two, np.float32).reshape(B, 96, HW)
    fp = np.asarray(flow_prev, np.float32)
    in_maps = []
    for i in range(N_CORES):
        m = dict(shared)
        f1t = (f1[i] * (1.0 / 96.0)).reshape(96, 6, 16, 20, 8)
        m["f1b"] = np.ascontiguousarray(
            f1t.transpose(0, 1, 3, 2, 4)).reshape(96, HW).astype(bf)
        ft = np.zeros((F2LEN, 128), bf)
        ft[PAD:PAD + HW, 0:96] = f2[i].T
        m["f2t"] = ft
        m["fp"] = fp[i]
        in_maps.append(m)
    return in_maps


def stage_inputs(in_maps, runner):
    """Concatenate per-core inputs on axis 0 and place on the 8 cores."""
    import jax
    args = []
    for nm in runner["in_names"]:
        args.append(np.concatenate([np.asarray(m[nm]) for m in in_maps],
                                   axis=0))
    for z in runner["zero_outs"]:
        args.append(np.zeros((N_CORES * z.shape[0], *z.shape[1:]), z.dtype))
    return [jax.device_put(a, runner["sharding"]) for a in args]


def run_staged(runner, dev_args):
    return runner["sharded"](*dev_args)


def kernel(feat_one, feat_two, flow_prev, up_w,
           w1, b1, w2, b2, w3, b3, w4, b4, debug=False):
    st = _get_state(debug)
    runner = st["runner"]
    in_maps = _build_in_maps(feat_one, feat_two, flow_prev, up_w,
                             w1, b1, w2, b2, w3, b3, w4, b4, st["consts"])
    dev_args = stage_inputs(in_maps, runner)
    outs = run_staged(runner, dev_args)
    oi = runner["out_names"].index("out")
    out = np.asarray(outs[oi]).reshape(N_CORES, 2, H, W).astype(np.float32)
    if debug:
        results = []
        for i in range(N_CORES):
            r = {}
            for j, nm in enumerate(runner["out_names"]):
                a = runner["out_avals"][j]
                r[nm] = np.asarray(outs[j]).reshape(N_CORES, *a.shape)[i]
            results.append(r)
        return out, results
    return out



# revision 39
# speedup vs baseline: 1.2999x; 1.1473x over previous
"""Trainium2 Bass kernel for nn_BasicVSR_LFN (upflow + backwarp + 7x7
correlation + 4 convs), data-parallel over batch: 1 sample per NeuronCore.

Per-core pipeline (shapes hardcoded for B=8, C=96, H=96, W=160):
  1. upflow (ConvTranspose2d 2->2, k4 s2 p1 groups=2) as 4 parity-plane
     stencils on DVE, assembled to flow[g][96,160] by strided DMA.
  2. warp coordinate/index/weight pipeline in [96y,160x] layout on DVE.
  3. idx -> interleaved int16 [96,960] (PE transpose + strided copies +
     partition-doubling DMA) for gpsimd ap_gather.
  4. backwarp: 4 ap_gather taps from padded f2 (AP offsets 0/1/160/161),
     bilinear combine on DVE; weights replicated across channel partitions
     by log-doubling DMAs per quarter. Output: padded channel-major warped
     bf16 [96, 102, 166].
  5. correlation: 120 PE matmuls (f1 16x8-pixel tile [96,128] x warped
     window [96,22,14]); PSUM [128,308] -> bf16 -> gpsimd local_scatter
     (static band table) -> [128,50] pixel-major -> PE pair transpose ->
     channel-major corr2 [100, 98, 163] (2x col-shift K-stack), lrelu.
  6. convs as K-packed shifted matmuls (N=480 row-aligned chunks), PSUM
     accumulation, fused bias+leaky-relu+bf16 cast via ACT Prelu copy-out.
     conv4 5x5 via 4-row-shift K-stack; out = flow + res.
"""
import numpy as np
import ml_dtypes

import concourse.bass as bass
import concourse.bacc as bacc
import concourse.mybir as mybir
import concourse.tile as tile
from contextlib import ExitStack

F32 = mybir.dt.float32
F32R = mybir.dt.float32r
BF16 = mybir.dt.float16  # fp16 everywhere (precision margin)
I16 = mybir.dt.int16
I32 = mybir.dt.int32
ALU = mybir.AluOpType
ACTF = mybir.ActivationFunctionType

B, C, H, W = 8, 96, 96, 160
HW = H * W
N_CORES = 8
PAD = 161
F2LEN = PAD + HW + PAD         # 15682
NEG = HW + PAD                 # 15521
AX = 2.5 * W / (W - 1.0)
AY = 2.5 * H / (H - 1.0)
EPSF = -0.5 + 2.0 ** -11

WPH, WPW = H + 6, W + 6        # 102, 166
CPH, CPW = H + 2, W + 3        # 98, 163
QPH, QPW = H + 4, W + 5        # 100, 165

GCH = 960
QP = 3840
NCH = 32                       # conv chunks (3 rows x 160)
CHP = 480

bf = np.float16

# upflow tap order (must match host table)
UP_ORDER = [(g, ry, rx, di, dj)
            for g in range(2) for ry in range(2) for rx in range(2)
            for di in ([-1, 0] if ry == 0 else [0, 1])
            for dj in ([-1, 0] if rx == 0 else [0, 1])]


def _host_consts():
    cs = {}
    cs["xg"] = np.tile(np.arange(W, dtype=np.float32)[None, :], (H, 1))
    cs["yg"] = np.tile(np.arange(H, dtype=np.float32)[:, None], (1, W))
    cs["id96"] = np.eye(96, dtype=np.float32)
    cs["id128b"] = np.eye(128, dtype=np.float32).astype(bf)
    lsi = np.full((128, 308), -1, dtype=np.int16)
    for m in range(128):
        r, c = m // 8, m % 8
        for n in range(308):
            Rr, Cc = n // 14, n % 14
            dy, dx = Rr - 3 - r, Cc - 3 - c
            if -3 <= dy <= 3 and -3 <= dx <= 3:
                lsi[m, n] = (dy + 3) * 7 + (dx + 3)
    cs["lsidx"] = lsi
    return cs


def _host_weights(up_w, w1, b1, w2, b2, w3, b3, w4, b4):
    ws = {}
    tab = np.zeros((48, 32), np.float32)
    for j, (g, ry, rx, di, dj) in enumerate(UP_ORDER):
        tab[:, j] = up_w[g, 0, 1 - 2 * di + ry, 1 - 2 * dj + rx]
    ws["upwtab"] = tab
    # conv1 pair [114, 3*128] (kx0 @ rows 0-48, kx1 @ rows 64-112),
    # single [50, 3*128]
    c1p = np.zeros((3, 114, 128), np.float32)
    c1s = np.zeros((3, 50, 128), np.float32)
    for ky in range(3):
        c1p[ky, 0:49] = w1[:, :, ky, 0].T
        c1p[ky, 64:113] = w1[:, :, ky, 1].T
        c1s[ky, 0:49] = w1[:, :, ky, 2].T
    ws["c1p"] = np.transpose(c1p, (1, 0, 2)).reshape(114, 384).astype(bf)
    ws["c1s"] = np.transpose(c1s, (1, 0, 2)).reshape(50, 384).astype(bf)
    c2 = np.zeros((9, 128, 64), np.float32)
    for ky in range(3):
        for kx in range(3):
            c2[ky * 3 + kx] = w2[:, :, ky, kx].T
    ws["c2"] = np.transpose(c2, (1, 0, 2)).reshape(128, 576).astype(bf)
    c3p = np.zeros((3, 128, 32), np.float32)
    c3s = np.zeros((3, 64, 32), np.float32)
    for ky in range(3):
        c3p[ky, 0:64] = w3[:, :, ky, 0].T
        c3p[ky, 64:128] = w3[:, :, ky, 1].T
        c3s[ky] = w3[:, :, ky, 2].T
    ws["c3p"] = np.transpose(c3p, (1, 0, 2)).reshape(128, 96).astype(bf)
    ws["c3s"] = np.transpose(c3s, (1, 0, 2)).reshape(64, 96).astype(bf)
    c4q = np.zeros((5, 128, 2), np.float32)
    c4s = np.zeros((5, 32, 2), np.float32)
    for kx in range(5):
        for dr in range(4):
            c4q[kx, dr * 32:(dr + 1) * 32] = w4[:, :, dr, kx].T
        c4s[kx] = w4[:, :, 4, kx].T
    ws["c4q"] = np.transpose(c4q, (1, 0, 2)).reshape(128, 10).astype(bf)
    ws["c4s"] = np.transpose(c4s, (1, 0, 2)).reshape(32, 10).astype(bf)
    # conv4 flow/bias injection matmul: out += I2 @ [flow; 1] rows + b4
    c4f = np.zeros((3, 2), np.float32)
    c4f[0, 0] = 1.0
    c4f[1, 1] = 1.0
    c4f[2, :] = np.asarray(b4, np.float32)
    ws["c4f"] = c4f
    for nm, b_ in (("b1", b1), ("b2", b2), ("b3", b3)):
        ws[nm] = np.asarray(b_, np.float32)[:, None]
    ws["al128"] = np.full((128, 1), 0.1, np.float32)
    ws["al64"] = np.full((64, 1), 0.1, np.float32)
    ws["al32"] = np.full((32, 1), 0.1, np.float32)
    return ws


def build_program(debug=False, sim_q0=False):
    # sim_q0: force all SWDGE gathers onto queue 0 — satisfies the
    # interp's lane/queue lock for local profiling; HW builds keep the
    # 4-queue assignment
    nc = bacc.Bacc("TRN2", target_bir_lowering=False, debug=False,
                   num_devices=N_CORES, num_swdge_queues=4)
    P = nc.declare_dram_parameter
    d_f1b = P("f1b", [96, HW], BF16, isOutput=False)
    d_f2t = P("f2t", [F2LEN, 128], BF16, isOutput=False)
    d_fp = P("fp", [2, 48, 80], F32, isOutput=False)
    d_upw = P("upwtab", [48, 32], F32, isOutput=False)
    d_xg = P("xg", [H, W], F32, isOutput=False)
    d_yg = P("yg", [H, W], F32, isOutput=False)
    d_id96 = P("id96", [96, 96], F32, isOutput=False)
    d_id128b = P("id128b", [128, 128], BF16, isOutput=False)
    d_lsi = P("lsidx", [128, 308], I16, isOutput=False)
    d_c1p = P("c1p", [114, 384], BF16, isOutput=False)
    d_c1s = P("c1s", [50, 384], BF16, isOutput=False)
    d_c2 = P("c2", [128, 576], BF16, isOutput=False)
    d_c3p = P("c3p", [128, 96], BF16, isOutput=False)
    d_c3s = P("c3s", [64, 96], BF16, isOutput=False)
    d_c4q = P("c4q", [128, 10], BF16, isOutput=False)
    d_c4s = P("c4s", [32, 10], BF16, isOutput=False)
    d_c4f = P("c4f", [3, 2], F32, isOutput=False)
    d_b1 = P("b1", [128, 1], F32, isOutput=False)
    d_b2 = P("b2", [64, 1], F32, isOutput=False)
    d_b3 = P("b3", [32, 1], F32, isOutput=False)
    d_al128 = P("al128", [128, 1], F32, isOutput=False)
    d_al64 = P("al64", [64, 1], F32, isOutput=False)
    d_al32 = P("al32", [32, 1], F32, isOutput=False)
    d_out = P("out", [2, H, W], F32, isOutput=True)
    d_wsc = nc.dram_tensor("wscratch", [4, HW], BF16)
    dbg = {}
    if debug:
        dbg["warped"] = P("dbg_warped", [96, WPH, WPW], BF16, isOutput=True)
        dbg["corr2"] = P("dbg_corr2", [114, CPH, CPW], BF16, isOutput=True)
        dbg["h1"] = P("dbg_h1", [128, CPH, CPW], BF16, isOutput=True)
        dbg["h3"] = P("dbg_h3", [128, QPH, QPW], BF16, isOutput=True)
        dbg["flow"] = P("dbg_flow", [2, H, W], F32, isOutput=True)
        dbg["il"] = P("dbg_il", [128, 960], I16, isOutput=True)

    with tile.TileContext(nc) as tc, ExitStack() as top:
        pc = top.enter_context(tc.tile_pool(name="pc", bufs=1))

        # ---- consts ----
        t_upw = pc.tile([48, 32], F32)
        t_id96 = pc.tile([96, 96], F32)
        t_id128b = pc.tile([128, 128], BF16)
        t_lsi = pc.tile([128, 308], I16)
        t_c1p = pc.tile([114, 384], BF16)
        t_c1s = pc.tile([50, 384], BF16)
        t_c2 = pc.tile([128, 576], BF16)
        t_c3p = pc.tile([128, 96], BF16)
        t_c3s = pc.tile([64, 96], BF16)
        t_c4q = pc.tile([128, 10], BF16)
        t_c4s = pc.tile([32, 10], BF16)
        t_c4f = pc.tile([3, 2], F32)
        t_b1 = pc.tile([128, 1], F32)
        t_b2 = pc.tile([64, 1], F32)
        t_b3 = pc.tile([32, 1], F32)
        t_al128 = pc.tile([128, 1], F32)
        t_al64 = pc.tile([64, 1], F32)
        t_al32 = pc.tile([32, 1], F32)
        for tt, dd in ((t_upw, d_upw),
                       (t_id96, d_id96), (t_id128b, d_id128b),
                       (t_lsi, d_lsi), (t_c1p, d_c1p), (t_c1s, d_c1s),
                       (t_c2, d_c2), (t_c3p, d_c3p), (t_c3s, d_c3s),
                       (t_c4q, d_c4q), (t_c4s, d_c4s), (t_c4f, d_c4f),
                       (t_b1, d_b1),
                       (t_b2, d_b2), (t_b3, d_b3),
                       (t_al128, d_al128), (t_al64, d_al64),
                       (t_al32, d_al32)):
            nc.sync.dma_start(tt[:], dd[:])

        # ---- upflow ----
        pp_cm = tc.tile_pool(name="pp", bufs=1)
        pp = pp_cm.__enter__()
        t_xg = pp.tile([H, W], F32)
        t_yg = pp.tile([H, W], F32)
        nc.sync.dma_start(t_xg[:], d_xg[:])
        nc.sync.dma_start(t_yg[:], d_yg[:])
        # fps[g][di+1]: flow_prev[g, p+di, q+dj] readable at col offset dj+1
        fps = {}
        for g in range(2):
            for di in (-1, 0, 1):
                nm = f"fps{g}_{di + 1}"
                t = pp.tile([48, 82], F32, tag=nm, name=nm)
                nc.vector.memset(t[:], 0.0)
                lo, hi = max(0, di), min(48, 48 + di)
                nc.sync.dma_start(t[lo - di:hi - di, 1:81], d_fp[g, lo:hi, :])
                fps[(g, di)] = t
        t_upt = pp.tile([48, 80], F32, tag="uptmp")
        planes = {}
        for key in {(g, ry, rx) for (g, ry, rx, _, _) in UP_ORDER}:
            nm = f"pl{key[0]}{key[1]}{key[2]}"
            planes[key] = pp.tile([48, 80], F32, tag=nm, name=nm)
        done = set()
        for j, (g, ry, rx, di, dj) in enumerate(UP_ORDER):
            pl = planes[(g, ry, rx)]
            sc = t_upw[:, j:j + 1]
            src = fps[(g, di)][:, 1 + dj:81 + dj]
            if (g, ry, rx) not in done:
                done.add((g, ry, rx))
                nc.vector.tensor_scalar(pl[:], src, sc, None, ALU.mult)
            else:
                nc.vector.tensor_scalar(t_upt[:], src, sc, None, ALU.mult)
                nc.vector.tensor_tensor(pl[:], pl[:], t_upt[:], ALU.add)
        t_flx = pc.tile([H, W], F32)
        t_fly = pc.tile([H, W], F32)
        flyx = [t_flx, t_fly]
        for (g, ry, rx), pl in sorted(planes.items()):
            nc.sync.dma_start(flyx[g][ry::2, rx::2], pl[:])

        # ---- warp index / weight pipeline ----
        def hwt(tag, dt=F32):
            return pp.tile([H, W], dt, tag=tag, name=tag)

        t_px, t_py = hwt("px"), hwt("py")
        nc.vector.tensor_scalar(t_px[:], t_flx[:], AX, None, ALU.mult)
        nc.vector.tensor_tensor(t_px[:], t_px[:], t_xg[:], ALU.add)
        nc.vector.tensor_scalar(t_py[:], t_fly[:], AY, None, ALU.mult)
        nc.vector.tensor_tensor(t_py[:], t_py[:], t_yg[:], ALU.add)
        t_x0, t_y0 = hwt("x0"), hwt("y0")
        t_i32 = pp.tile([H, W], I32, tag="i32")
        nc.vector.tensor_scalar(t_x0[:], t_px[:], EPSF, None, ALU.add)
        nc.vector.tensor_copy(t_i32[:], t_x0[:])
        nc.vector.tensor_copy(t_x0[:], t_i32[:])
        nc.vector.tensor_scalar(t_y0[:], t_py[:], EPSF, None, ALU.add)
        nc.vector.tensor_copy(t_i32[:], t_y0[:])
        nc.vector.tensor_copy(t_y0[:], t_i32[:])
        t_wx1, t_wy1, t_wx0, t_wy0 = (hwt("wx1"), hwt("wy1"),
                                      hwt("wx0"), hwt("wy0"))
        nc.vector.tensor_tensor(t_wx1[:], t_px[:], t_x0[:], ALU.subtract)
        nc.vector.tensor_tensor(t_wy1[:], t_py[:], t_y0[:], ALU.subtract)
        nc.vector.tensor_scalar(t_wx0[:], t_wx1[:], -1.0, 1.0, ALU.mult, ALU.add)
        nc.vector.tensor_scalar(t_wy0[:], t_wy1[:], -1.0, 1.0, ALU.mult, ALU.add)
        t_m1, t_m2 = hwt("m1"), hwt("m2")
        wviews = {}
        for nm, t_base, t_w, lo, hi in (
                ("wx0", t_x0, t_wx0, 0.0, float(W - 1)),
                ("wx1", t_x0, t_wx1, -1.0, float(W - 2)),
                ("wy0", t_y0, t_wy0, 0.0, float(H - 1)),
                ("wy1", t_y0, t_wy1, -1.0, float(H - 2))):
            nc.vector.tensor_scalar(t_m1[:], t_base[:], lo, None, ALU.is_ge)
            nc.vector.tensor_scalar(t_m2[:], t_base[:], hi, None, ALU.is_le)
            nc.vector.tensor_tensor(t_m1[:], t_m1[:], t_m2[:], ALU.mult)
            wv = pc.tile([H, W], BF16, tag=f"wv{nm}")
            nc.vector.tensor_tensor(wv[:], t_w[:], t_m1[:], ALU.mult)
            wviews[nm] = wv
        # fuse the x/y weights into the 4 per-tap products w_ij =
        # wx_j * wy_i (kills the separate y-multiply stage in the warp
        # combine), then flatten each [H,W] plane to a d_wsc row
        wprod = {}
        for wi, (nm, nx, ny) in enumerate(
                (("w00", "wx0", "wy0"), ("w01", "wx1", "wy0"),
                 ("w10", "wx0", "wy1"), ("w11", "wx1", "wy1"))):
            wp = pc.tile([H, W], BF16, tag=f"wp{nm}")
            nc.vector.tensor_tensor(wp[:], wviews[nx][:], wviews[ny][:],
                                    ALU.mult)
            wprod[nm] = wp
            nc.sync.dma_start(d_wsc[wi:wi + 1, :], wp[:])
        t_idx = hwt("idxf")
        nc.vector.tensor_scalar(t_m1[:], t_y0[:], -1.0, float(H - 1),
                                ALU.max, ALU.min)
        nc.vector.tensor_scalar(t_m2[:], t_x0[:], -1.0, float(W - 1),
                                ALU.max, ALU.min)
        nc.vector.tensor_scalar(t_idx[:], t_m1[:], float(W), float(PAD),
                                ALU.mult, ALU.add)
        nc.vector.tensor_tensor(t_idx[:], t_idx[:], t_m2[:], ALU.add)

        # ---- interleave idx -> il [96, 960] int16 ----
        t_il = pc.tile([128, 960], I16)
        with tc.tile_pool(name="ps_tr", bufs=2,
                          space=bass.MemorySpace.PSUM) as ps_tr:
            for u in range(10):
                p_t = ps_tr.tile([16, 96], F32, tag="pt", name="p_t")
                nc.tensor.transpose(p_t[:], t_idx[:, 16 * u:16 * u + 16],
                                    t_id96[:])
                nc.vector.tensor_copy(t_il[0:16, u::10], p_t[:])
        k = 16
        while k < 128:
            n = min(k, 128 - k)
            nc.sync.dma_start(t_il[k:k + n, :], t_il[0:n, :])
            k += n
        if debug:
            nc.sync.dma_start(dbg["il"][:], t_il[:])
        pp_cm.__exit__(None, None, None)

        pcv = top.enter_context(tc.tile_pool(name="pcv", bufs=1))
        t_corr2 = pcv.tile([114, CPH, CPW], BF16)

        # ---- warp phase ----
        pmA_cm = tc.tile_pool(name="pmA", bufs=1)
        pmA = pmA_cm.__enter__()
        t_warp = pmA.tile([96, WPH, WPW], BF16)
        nc.vector.memset(t_warp[:, 0:3, :], 0.0)
        nc.vector.memset(t_warp[:, WPH - 3:WPH, :], 0.0)
        nc.vector.memset(t_warp[:, 3:WPH - 3, 0:3], 0.0)
        nc.vector.memset(t_warp[:, 3:WPH - 3, WPW - 3:WPW], 0.0)

        # per quarter: 2 paired gathers x 6 chunks (elem 256 @ step 128
        # fetches the x/x+1 taps together), 4 weight-product broadcasts,
        # in-place bilinear combine.
        #
        # Gathers spread over all 4 SWDGE queues. Soundness: the auto
        # DMASW lane sems are shared round-robin with cumulative counts,
        # so a LATER quarter's completion on another queue could satisfy
        # an earlier quarter's wait threshold while its own data is in
        # flight. A barrier (csem) gating quarter q+1's gathers on
        # quarter q's combine keeps every increment a reader counts
        # inside its own dependency set, for any queue assignment.
        csem = nc.alloc_semaphore("warp_comb")
        nc.gpsimd.sem_clear(csem)
        with tc.tile_pool(name="pw", bufs=2) as pw, \
                tc.tile_pool(name="pg", bufs=2) as pg:
            for q in range(4):
                r0 = q * 24
                reps = {}
                for wi, nm in enumerate(("w00", "w01", "w10", "w11")):
                    rep = pw.tile([96, QP], BF16, tag=f"rp{nm}", name=nm)
                    src = d_wsc[wi:wi + 1, q * QP:(q + 1) * QP]
                    rsrc = bass.AP(src.tensor, src.offset,
                                   [[0, 96]] + list(src.ap)[1:])
                    nc.sync.dma_start(rep[:], rsrc)
                    reps[nm] = rep
                if q >= 1:
                    nc.gpsimd.wait_ge(csem, q)
                gA = pg.tile([128, 6, 2, 640], BF16, tag="gA")
                gB = pg.tile([128, 6, 2, 640], BF16, tag="gB")
                for c6 in range(6):
                    for ti, (gt, off) in enumerate(((gA, 0), (gB, W))):
                        src = bass.AP(d_f2t[:].tensor,
                                      d_f2t[:].offset + off * 128,
                                      [[128, NEG], [1, 256]])
                        nc.gpsimd.dma_gather(
                            gt[:, c6, :, :], src,
                            t_il[:, 240 * q + 40 * c6:240 * q + 40 * (c6 + 1)],
                            num_idxs=640, num_idxs_reg=640, elem_size=256,
                            elem_step=128, transpose=True,
                            queue_num=0 if sim_q0 else (2 * c6 + ti) % 4)
                # chunk = 640 px = exactly 4 warp rows, so the strided
                # [96, 6, 4, 160] views line up with the padded t_warp
                gA0 = gA[0:96, :, 0, :]
                gA1 = gA[0:96, :, 1, :]
                gB0 = gB[0:96, :, 0, :]
                gB1 = gB[0:96, :, 1, :]
                rw = {nm: reps[nm][:].rearrange("p (c n) -> p c n", c=6)
                      for nm in reps}
                nc.vector.tensor_tensor(gA0, gA0, rw["w00"], ALU.mult)
                nc.vector.tensor_tensor(gA1, gA1, rw["w01"], ALU.mult)
                nc.vector.tensor_tensor(gA0, gA0, gA1, ALU.add)
                nc.vector.tensor_tensor(gB0, gB0, rw["w10"], ALU.mult)
                nc.vector.tensor_tensor(gB1, gB1, rw["w11"], ALU.mult)
                nc.vector.tensor_tensor(gB0, gB0, gB1, ALU.add)
                wdst = t_warp[:, 3 + r0:3 + r0 + 24, 3:3 + W]
                wdst = wdst.rearrange("p (c r) w -> p c r w", r=4)
                nc.vector.tensor_tensor(
                    wdst, gA0.rearrange("p c (r w) -> p c r w", w=W),
                    gB0.rearrange("p c (r w) -> p c r w", w=W), ALU.add)
                # DVE is in-order: this inc certifies the whole combine
                # (hence all 12 gathers of this quarter) is done
                nc.vector.sem_inc(csem, 1)

        if debug:
            nc.sync.dma_start(dbg["warped"][:], t_warp[:])
            nc.sync.dma_start(dbg["flow"][0], t_flx[:])
            nc.sync.dma_start(dbg["flow"][1], t_fly[:])

        # ---- correlation ----
        nc.vector.memset(t_corr2[32:64], 0.0)
        nc.vector.memset(t_corr2[:, 0:1, :], 0.0)
        nc.vector.memset(t_corr2[:, CPH - 1:CPH, :], 0.0)
        nc.vector.memset(t_corr2[:, 1:CPH - 1, 0:1], 0.0)
        nc.vector.memset(t_corr2[:, 1:CPH - 1, W + 1:CPW], 0.0)

        pf1_cm = tc.tile_pool(name="pf1", bufs=1)
        pf1 = pf1_cm.__enter__()
        t_f1b = pf1.tile([96, HW], BF16)
        # band-wise loads: band b's matmuls only wait on chunk b
        for b6 in range(6):
            nc.sync.dma_start(t_f1b[:, b6 * 2560:(b6 + 1) * 2560],
                              d_f1b[:, b6 * 2560:(b6 + 1) * 2560])

        with tc.tile_pool(name="pcr", bufs=4) as pcr, \
                tc.tile_pool(name="pst", bufs=4) as pst, \
                tc.tile_pool(name="ps_c", bufs=4,
                             space=bass.MemorySpace.PSUM) as ps_c, \
                tc.tile_pool(name="ps_p", bufs=2,
                             space=bass.MemorySpace.PSUM) as ps_p:
            for band in range(6):
                Y = band * 16
                for grp in range(3):
                    npair = 4 if grp < 2 else 2
                    p_pa = ps_p.tile([50, 512], BF16, tag="packa")
                    p_pb = ps_p.tile([50, 512], BF16, tag="packb")
                    for pj in range(npair):
                        stk = pst.tile([128, 100], BF16, tag="stk")
                        for half in range(2):
                            tx = grp * 8 + pj * 2 + half
                            X = tx * 8
                            p_c = ps_c.tile([128, 308], F32, tag="pcorr")
                            ti128 = (band * 20 + tx) * 128
                            nc.tensor.matmul(
                                p_c[:], t_f1b[:, ti128:ti128 + 128],
                                t_warp[:, Y:Y + 22, X:X + 14],
                                start=True, stop=True)
                            sb = pcr.tile([128, 308], BF16, tag="sbc")
                            if half == 0:
                                nc.vector.tensor_copy(sb[:], p_c[:])
                            else:
                                nc.scalar.activation(sb[:], p_c[:], ACTF.Copy)
                            nc.gpsimd.local_scatter(
                                stk[:, half * 50:half * 50 + 50], sb[:],
                                t_lsi[:], channels=128, num_elems=50,
                                num_idxs=308)
                        nc.tensor.transpose(
                            p_pa[:, pj * 128:(pj + 1) * 128], stk[:, 0:50],
                            t_id128b[:])
                        nc.tensor.transpose(
                            p_pb[:, pj * 128:(pj + 1) * 128], stk[:, 50:100],
                            t_id128b[:])
                    for half in range(2):
                        xbase = grp * 64 + half * 8
                        cv = t_corr2[:]
                        shp = [[CPH * CPW, 50], [16, npair], [CPW, 16], [1, 8]]
                        dst = bass.AP(
                            cv.tensor,
                            cv.offset + (1 + Y) * CPW + (1 + xbase), shp)
                        # kx1-tap duplicate: rows 64:114, cols shifted by -1
                        dst2 = bass.AP(
                            cv.tensor,
                            cv.offset + 64 * (CPH * CPW) + (1 + Y) * CPW
                            + xbase, shp)
                        src = (p_pa if half == 0 else p_pb)[:]
                        src = src.rearrange("p (j r c) -> p j r c", r=16, c=8)
                        src = src[:, 0:npair]
                        # fused lrelu on copy-out (ACT), then partition-
                        # shifted duplicate for the conv1 kx1 K-pack (DVE)
                        nc.scalar.activation(dst, src, ACTF.Prelu,
                                             bias=0.0, scale=1.0,
                                             alpha=t_al128[0:50])
                        nc.vector.tensor_copy(dst2, dst)

        pf1_cm.__exit__(None, None, None)
        pmA_cm.__exit__(None, None, None)

        if debug:
            nc.sync.dma_start(dbg["corr2"][:], t_corr2[:])

        # ---- convs ----
        pcv2 = top.enter_context(tc.tile_pool(name="pcv2", bufs=1))
        t_h1 = pcv2.tile([128, CPH, CPW], BF16)
        nc.vector.memset(t_h1[:, 0:1, :], 0.0)
        nc.vector.memset(t_h1[:, CPH - 1:CPH, :], 0.0)
        nc.vector.memset(t_h1[:, 1:CPH - 1, 0:1], 0.0)
        nc.vector.memset(t_h1[:, 1:CPH - 1, W + 1:CPW], 0.0)

        with tc.tile_pool(name="ps_cv", bufs=4,
                          space=bass.MemorySpace.PSUM) as ps_cv:
            # conv1
            for ch in range(NCH):
                r = 3 * ch
                p_o = ps_cv.tile([128, CHP], F32, tag="cvo")
                for ky in range(3):
                    nc.tensor.matmul(
                        p_o[:], t_c1p[:, ky * 128:(ky + 1) * 128],
                        t_corr2[0:114, r + ky:r + ky + 3, 0:W],
                        start=(ky == 0), stop=False)
                    nc.tensor.matmul(
                        p_o[:], t_c1s[:, ky * 128:(ky + 1) * 128],
                        t_corr2[0:50, r + ky:r + ky + 3, 2:2 + W],
                        start=False, stop=(ky == 2))
                nc.scalar.activation(
                    t_h1[:, r + 1:r + 4, 1:1 + W],
                    p_o[:].rearrange("p (r w) -> p r w", w=W),
                    ACTF.Prelu, bias=t_b1[:], scale=1.0, alpha=t_al128[:])
            if debug:
                nc.sync.dma_start(dbg["h1"][:], t_h1[:])

            # conv2 — chunk pairs run concurrently on the two 64-col PE
            # tiles (tile_position derives from the psum partition base)
            t_h2 = pcv2.tile([128, CPH, CPW], BF16)
            nc.vector.memset(t_h2[:, 0:1, :], 0.0)
            nc.vector.memset(t_h2[:, CPH - 1:CPH, :], 0.0)
            nc.vector.memset(t_h2[:, 1:CPH - 1, 0:1], 0.0)
            nc.vector.memset(t_h2[:, 1:CPH - 1, W + 1:CPW], 0.0)
            for pch in range(0, NCH, 2):
                p_o = ps_cv.tile([128, CHP], F32, tag="cvo")
                for ti in range(9):
                    ky, kx = divmod(ti, 3)
                    for half in range(2):
                        r = 3 * (pch + half)
                        nc.tensor.matmul(
                            p_o[64 * half:64 * half + 64],
                            t_c2[:, ti * 64:(ti + 1) * 64],
                            t_h1[:, r + ky:r + ky + 3, kx:kx + W],
                            start=(ti == 0), stop=(ti == 8),
                            skip_group_check=True,
                            tile_position=(0, 64 * half))
                for half in range(2):
                    r = 3 * (pch + half)
                    nc.scalar.activation(
                        t_h2[0:64, r + 1:r + 4, 1:1 + W],
                        p_o[64 * half:64 * half + 64].rearrange(
                            "p (r w) -> p r w", w=W),
                        ACTF.Prelu, bias=t_b2[:], scale=1.0, alpha=t_al64[:])
                    # kx1-tap duplicate for conv3 K-pack: partitions
                    # 64:128, cols shifted by -1 (DVE, overlaps matmuls)
                    nc.vector.tensor_copy(
                        t_h2[64:128, r + 1:r + 4, 0:W],
                        t_h2[0:64, r + 1:r + 4, 1:1 + W])

            # conv3 -> h3 (padded 100x165 @ (2,2)); the 3 row-shifted
            # ky planes for conv4's K-pack are built chunk-by-chunk with
            # partition-shifted engine copies that overlap the matmuls
            t_h3 = pcv2.tile([128, QPH, QPW], BF16)
            nc.vector.memset(t_h3[0:32, 0:2, :], 0.0)
            nc.vector.memset(t_h3[0:32, QPH - 2:QPH, :], 0.0)
            nc.vector.memset(t_h3[:, 2:QPH - 2, 0:2], 0.0)
            nc.vector.memset(t_h3[:, 2:QPH - 2, W + 2:QPW], 0.0)
            nc.vector.memset(t_h3[32:64, 0:2, :], 0.0)
            nc.vector.memset(t_h3[64:128, 0:2, :], 0.0)
            nc.vector.memset(t_h3[96:128, 95:96, :], 0.0)
            # chunk quads run concurrently on the four 32-col PE tiles
            for qch in range(0, NCH, 4):
                p_o = ps_cv.tile([128, CHP], F32, tag="cvo")
                for ky in range(3):
                    for m in range(4):
                        r = 3 * (qch + m)
                        nc.tensor.matmul(
                            p_o[32 * m:32 * m + 32],
                            t_c3p[:, ky * 32:(ky + 1) * 32],
                            t_h2[0:128, r + ky:r + ky + 3, 0:W],
                            start=(ky == 0), stop=False,
                            skip_group_check=True,
                            tile_position=(0, 32 * m))
                    for m in range(4):
                        r = 3 * (qch + m)
                        nc.tensor.matmul(
                            p_o[32 * m:32 * m + 32],
                            t_c3s[:, ky * 32:(ky + 1) * 32],
                            t_h2[0:64, r + ky:r + ky + 3, 2:2 + W],
                            start=False, stop=(ky == 2),
                            skip_group_check=True,
                            tile_position=(0, 32 * m))
                for m in range(4):
                    r = 3 * (qch + m)
                    nc.scalar.activation(
                        t_h3[0:32, r + 2:r + 5, 2:2 + W],
                        p_o[32 * m:32 * m + 32].rearrange(
                            "p (r w) -> p r w", w=W),
                        ACTF.Prelu, bias=t_b3[:], scale=1.0, alpha=t_al32[:])
                    for dr, eng in ((1, nc.vector), (2, nc.gpsimd),
                                    (3, nc.vector)):
                        lo = max(0, r + 2 - dr)
                        hi = r + 5 - dr
                        eng.tensor_copy(
                            t_h3[32 * dr:32 * dr + 32, lo:hi, 2:2 + W],
                            t_h3[0:32, lo + dr:hi + dr, 2:2 + W])
            if debug:
                nc.sync.dma_start(dbg["h3"][:], t_h3[:])

            # conv4: chunk quads on the four 32-col PE tiles; flow + bias
            # accumulated into PSUM via a K=3 identity/bias matmul (f32r)
            with tc.tile_pool(name="po4", bufs=3) as po4:
                for qg in range(4):
                    t_fl3 = po4.tile([3, QP], F32, tag="flfq", bufs=2)
                    rq = qg * 24
                    # engine memsets must start at partition 0/32/64/96:
                    # fill all 3 rows with 1.0, then overwrite 0:2 w/ flow
                    nc.vector.memset(t_fl3[0:3, :], 1.0)
                    nc.sync.dma_start(t_fl3[0:1, :], t_flx[rq:rq + 24, :])
                    nc.sync.dma_start(t_fl3[1:2, :], t_fly[rq:rq + 24, :])
                    t_oq = po4.tile([2, QP], F32, tag="oq", bufs=2)
                    for qc in range(0, 8, 4):
                        p_o = ps_cv.tile([128, CHP], F32, tag="cvo")
                        for kx in range(5):
                            for m in range(4):
                                r = 3 * (qg * 8 + qc + m)
                                nc.tensor.matmul(
                                    p_o[32 * m:32 * m + 2],
                                    t_c4q[:, kx * 2:kx * 2 + 2],
                                    t_h3[0:128, r:r + 3, kx:kx + W],
                                    start=(kx == 0), stop=False,
                                    skip_group_check=True,
                                    tile_position=(0, 32 * m))
                            for m in range(4):
                                r = 3 * (qg * 8 + qc + m)
                                nc.tensor.matmul(
                                    p_o[32 * m:32 * m + 2],
                                    t_c4s[:, kx * 2:kx * 2 + 2],
                                    t_h3[0:32, r + 4:r + 7, kx:kx + W],
                                    start=False, stop=False,
                                    skip_group_check=True,
                                    tile_position=(0, 32 * m))
                        for m in range(4):
                            cc = qc + m
                            nc.tensor.matmul(
                                p_o[32 * m:32 * m + 2],
                                t_c4f[:],
                                t_fl3[:, cc * CHP:(cc + 1) * CHP],
                                start=False, stop=True,
                                skip_group_check=True,
                                tile_position=(0, 32 * m))
                        for m in range(4):
                            cc = qc + m
                            nc.vector.tensor_copy(
                                t_oq[:, cc * CHP:(cc + 1) * CHP],
                                p_o[32 * m:32 * m + 2])
                    nc.sync.dma_start(
                        d_out[:, rq:rq + 24, :],
                        t_oq[:].rearrange("p (r w) -> p r w", w=W))

    nc.compile()
    return nc


_STATE = {}


def _make_runner(nc):
    """Build a persistent jitted shard_map callable for the compiled Bass
    module (mirrors bass2jax.run_bass_via_pjrt, but reusable + exposes
    device placement for steady-state timing)."""
    import jax
    import numpy as _np
    from jax.sharding import Mesh, PartitionSpec, NamedSharding
    from jax.experimental.shard_map import shard_map
    from concourse import bass2jax as b2j
    from concourse import mybir as _mb

    b2j.install_neuronx_cc_hook()
    partition_name = (nc.partition_id_tensor.name
                      if nc.partition_id_tensor else None)
    in_names, out_names, out_avals, zero_outs = [], [], [], []
    for alloc in nc.m.functions[0].allocations:
        if not isinstance(alloc, _mb.MemoryLocationSet):
            continue
        name = alloc.memorylocations[0].name
        if alloc.kind == "ExternalInput":
            if name != partition_name:
                in_names.append(name)
        elif alloc.kind == "ExternalOutput":
            shape = tuple(alloc.tensor_shape)
            dtype = _mb.dt.np(alloc.dtype)
            out_names.append(name)
            out_avals.append(jax.core.ShapedArray(shape, dtype))
            zero_outs.append(_np.zeros(shape, dtype))
    n_params = len(in_names)
    all_in = list(in_names) + list(out_names)
    if partition_name is not None:
        all_in.append(partition_name)

    def _body(*args):
        operands = list(args)
        if partition_name is not None:
            operands.append(b2j.partition_id_tensor())
        outs = b2j._bass_exec_p.bind(
            *operands,
            out_avals=tuple(out_avals),
            in_names=tuple(all_in),
            out_names=tuple(out_names),
            lowering_input_output_aliases=(),
            sim_require_finite=True,
            sim_require_nnan=True,
            nc=nc,
        )
        return tuple(outs)

    devices = jax.devices()[:N_CORES]
    mesh = Mesh(np.asarray(devices), ("core",))
    nsh = len(in_names) + len(out_names)
    sharded = jax.jit(
        shard_map(_body, mesh=mesh,
                  in_specs=(PartitionSpec("core"),) * nsh,
                  out_specs=(PartitionSpec("core"),) * len(out_names),
                  check_rep=False),
        keep_unused=True)
    sharding = NamedSharding(mesh, PartitionSpec("core"))
    return {
        "in_names": in_names, "out_names": out_names,
        "zero_outs": zero_outs, "sharded": sharded, "sharding": sharding,
        "out_avals": out_avals,
    }


def _get_state(debug=False):
    key = "dbg" if debug else "main"
    if key not in _STATE:
        nc = build_program(debug=debug)
        _STATE[key] = {"nc": nc, "consts": _host_consts(),
                       "runner": _make_runner(nc)}
    return _STATE[key]


def _build_in_maps(feat_one, feat_two, flow_prev, up_w,
                   w1, b1, w2, b2, w3, b3, w4, b4, consts):
    ws = _host_weights(np.asarray(up_w, np.float32),
                       np.asarray(w1, np.float32), np.asarray(b1, np.float32),
                       np.asarray(w2, np.float32), np.asarray(b2, np.float32),
                       np.asarray(w3, np.float32), np.asarray(b3, np.float32),
                       np.asarray(w4, np.float32), np.asarray(b4, np.float32))
    shared = {"xg": consts["xg"], "yg": consts["yg"], "id96": consts["id96"],
              "id128b": consts["id128b"], "lsidx": consts["lsidx"]}
    for nm in ("upwtab", "c1p", "c1s", "c2", "c3p", "c3s", "c4q", "c4s",
               "c4f", "b1", "b2", "b3", "al128", "al64", "al32"):
        shared[nm] = ws[nm]
    f1 = np.asarray(feat_one, np.float32).reshape(B, 96, HW)
    f2 = np.asarray(feat_two, np.float32).reshape(B, 96, HW)
    fp = np.asarray(flow_prev, np.float32)
    in_maps = []
    for i in range(N_CORES):
        m = dict(shared)
        f1t = (f1[i] * (1.0 / 96.0)).reshape(96, 6, 16, 20, 8)
        m["f1b"] = np.ascontiguousarray(
            f1t.transpose(0, 1, 3, 2, 4)).reshape(96, HW).astype(bf)
        ft = np.zeros((F2LEN, 128), bf)
        ft[PAD:PAD + HW, 0:96] = f2[i].T
        m["f2t"] = ft
        m["fp"] = fp[i]
        in_maps.append(m)
    return in_maps


def stage_inputs(in_maps, runner):
    """Concatenate per-core inputs on axis 0 and place on the 8 cores."""
    import jax
    args = []
    for nm in runner["in_names"]:
        args.append(np.concatenate([np.asarray(m[nm]) for m in in_maps],
                                   axis=0))
    for z in runner["zero_outs"]:
        args.append(np.zeros((N_CORES * z.shape[0], *z.shape[1:]), z.dtype))
    return [jax.device_put(a, runner["sharding"]) for a in args]


def run_staged(runner, dev_args):
    return runner["sharded"](*dev_args)


def kernel(feat_one, feat_two, flow_prev, up_w,
           w1, b1, w2, b2, w3, b3, w4, b4, debug=False):
    st = _get_state(debug)
    runner = st["runner"]
    in_maps = _build_in_maps(feat_one, feat_two, flow_prev, up_w,
                             w1, b1, w2, b2, w3, b3, w4, b4, st["consts"])
    dev_args = stage_inputs(in_maps, runner)
    outs = run_staged(runner, dev_args)
    oi = runner["out_names"].index("out")
    out = np.asarray(outs[oi]).reshape(N_CORES, 2, H, W).astype(np.float32)
    if debug:
        results = []
        for i in range(N_CORES):
            r = {}
            for j, nm in enumerate(runner["out_names"]):
                a = runner["out_avals"][j]
                r[nm] = np.asarray(outs[j]).reshape(N_CORES, *a.shape)[i]
            results.append(r)
        return out, results
    return out



# revision 40
# speedup vs baseline: 1.4461x; 1.1124x over previous
"""Trainium2 Bass kernel for nn_BasicVSR_LFN (upflow + backwarp + 7x7
correlation + 4 convs), data-parallel over batch: 1 sample per NeuronCore.

Per-core pipeline (shapes hardcoded for B=8, C=96, H=96, W=160):
  1. upflow (ConvTranspose2d 2->2, k4 s2 p1 groups=2) as 4 parity-plane
     stencils on DVE, assembled to flow[g][96,160] by strided DMA.
  2. warp coordinate/index/weight pipeline in [96y,160x] layout on DVE.
  3. idx -> interleaved int16 [96,960] (PE transpose + strided copies +
     partition-doubling DMA) for gpsimd ap_gather.
  4. backwarp: 4 ap_gather taps from padded f2 (AP offsets 0/1/160/161),
     bilinear combine on DVE; weights replicated across channel partitions
     by log-doubling DMAs per quarter. Output: padded channel-major warped
     bf16 [96, 102, 166].
  5. correlation: 120 PE matmuls (f1 16x8-pixel tile [96,128] x warped
     window [96,22,14]); PSUM [128,308] -> bf16 -> gpsimd local_scatter
     (static band table) -> [128,50] pixel-major -> PE pair transpose ->
     channel-major corr2 [100, 98, 163] (2x col-shift K-stack), lrelu.
  6. convs as K-packed shifted matmuls (N=480 row-aligned chunks), PSUM
     accumulation, fused bias+leaky-relu+bf16 cast via ACT Prelu copy-out.
     conv4 5x5 via 4-row-shift K-stack; out = flow + res.
"""
import numpy as np
import ml_dtypes

import concourse.bass as bass
import concourse.bacc as bacc
import concourse.mybir as mybir
import concourse.tile as tile
from contextlib import ExitStack

F32 = mybir.dt.float32
F32R = mybir.dt.float32r
BF16 = mybir.dt.float16  # fp16 everywhere (precision margin)
I16 = mybir.dt.int16
I32 = mybir.dt.int32
ALU = mybir.AluOpType
ACTF = mybir.ActivationFunctionType

B, C, H, W = 8, 96, 96, 160
HW = H * W
N_CORES = 8
PAD = 161
F2LEN = PAD + HW + PAD         # 15682
NEG = HW + PAD                 # 15521
AX = 2.5 * W / (W - 1.0)
AY = 2.5 * H / (H - 1.0)
EPSF = -0.5 + 2.0 ** -11

WPH, WPW = H + 6, W + 6        # 102, 166
CPH, CPW = H + 2, W + 3        # 98, 163
QPH, QPW = H + 4, W + 5        # 100, 165

GCH = 960
QP = 3840
NCH = 32                       # conv chunks (3 rows x 160)
CHP = 480

bf = np.float16

# upflow tap order (must match host table)
UP_ORDER = [(g, ry, rx, di, dj)
            for g in range(2) for ry in range(2) for rx in range(2)
            for di in ([-1, 0] if ry == 0 else [0, 1])
            for dj in ([-1, 0] if rx == 0 else [0, 1])]


def _host_consts():
    cs = {}
    cs["xg"] = np.tile(np.arange(W, dtype=np.float32)[None, :], (H, 1))
    cs["yg"] = np.tile(np.arange(H, dtype=np.float32)[:, None], (1, W))
    cs["id96"] = np.eye(96, dtype=np.float32)
    cs["id128b"] = np.eye(128, dtype=np.float32).astype(bf)
    lsi = np.full((128, 308), -1, dtype=np.int16)
    for m in range(128):
        r, c = m // 8, m % 8
        for n in range(308):
            Rr, Cc = n // 14, n % 14
            dy, dx = Rr - 3 - r, Cc - 3 - c
            if -3 <= dy <= 3 and -3 <= dx <= 3:
                lsi[m, n] = (dy + 3) * 7 + (dx + 3)
    cs["lsidx"] = lsi
    return cs


def _host_weights(up_w, w1, b1, w2, b2, w3, b3, w4, b4):
    ws = {}
    tab = np.zeros((48, 32), np.float32)
    for j, (g, ry, rx, di, dj) in enumerate(UP_ORDER):
        tab[:, j] = up_w[g, 0, 1 - 2 * di + ry, 1 - 2 * dj + rx]
    ws["upwtab"] = tab
    # conv1 pair [114, 3*128] (kx0 @ rows 0-48, kx1 @ rows 64-112),
    # single [50, 3*128]
    c1p = np.zeros((3, 114, 128), np.float32)
    c1s = np.zeros((3, 50, 128), np.float32)
    for ky in range(3):
        c1p[ky, 0:49] = w1[:, :, ky, 0].T
        c1p[ky, 64:113] = w1[:, :, ky, 1].T
        c1s[ky, 0:49] = w1[:, :, ky, 2].T
    ws["c1p"] = np.transpose(c1p, (1, 0, 2)).reshape(114, 384).astype(bf)
    ws["c1s"] = np.transpose(c1s, (1, 0, 2)).reshape(50, 384).astype(bf)
    c2 = np.zeros((9, 128, 64), np.float32)
    for ky in range(3):
        for kx in range(3):
            c2[ky * 3 + kx] = w2[:, :, ky, kx].T
    ws["c2"] = np.transpose(c2, (1, 0, 2)).reshape(128, 576).astype(bf)
    c3p = np.zeros((3, 128, 32), np.float32)
    c3s = np.zeros((3, 64, 32), np.float32)
    for ky in range(3):
        c3p[ky, 0:64] = w3[:, :, ky, 0].T
        c3p[ky, 64:128] = w3[:, :, ky, 1].T
        c3s[ky] = w3[:, :, ky, 2].T
    ws["c3p"] = np.transpose(c3p, (1, 0, 2)).reshape(128, 96).astype(bf)
    ws["c3s"] = np.transpose(c3s, (1, 0, 2)).reshape(64, 96).astype(bf)
    c4q = np.zeros((5, 128, 2), np.float32)
    c4s = np.zeros((5, 32, 2), np.float32)
    for kx in range(5):
        for dr in range(4):
            c4q[kx, dr * 32:(dr + 1) * 32] = w4[:, :, dr, kx].T
        c4s[kx] = w4[:, :, 4, kx].T
    ws["c4q"] = np.transpose(c4q, (1, 0, 2)).reshape(128, 10).astype(bf)
    ws["c4s"] = np.transpose(c4s, (1, 0, 2)).reshape(32, 10).astype(bf)
    # conv4 flow/bias injection matmul: out += I2 @ [flow; 1] rows + b4
    c4f = np.zeros((3, 2), np.float32)
    c4f[0, 0] = 1.0
    c4f[1, 1] = 1.0
    c4f[2, :] = np.asarray(b4, np.float32)
    ws["c4f"] = c4f
    for nm, b_ in (("b1", b1), ("b2", b2), ("b3", b3)):
        ws[nm] = np.asarray(b_, np.float32)[:, None]
    ws["al128"] = np.full((128, 1), 0.1, np.float32)
    ws["al64"] = np.full((64, 1), 0.1, np.float32)
    ws["al32"] = np.full((32, 1), 0.1, np.float32)
    return ws


def build_program(debug=False, sim_q0=False):
    # sim_q0: force all SWDGE gathers onto queue 0 — satisfies the
    # interp's lane/queue lock for local profiling; HW builds keep the
    # 4-queue assignment
    nc = bacc.Bacc("TRN2", target_bir_lowering=False, debug=False,
                   num_devices=N_CORES, num_swdge_queues=4)
    P = nc.declare_dram_parameter
    d_f1b = P("f1b", [96, HW], BF16, isOutput=False)
    d_f2t = P("f2t", [F2LEN, 128], BF16, isOutput=False)
    d_fp = P("fp", [2, 48, 80], F32, isOutput=False)
    d_upw = P("upwtab", [48, 32], F32, isOutput=False)
    d_xg = P("xg", [H, W], F32, isOutput=False)
    d_yg = P("yg", [H, W], F32, isOutput=False)
    d_id96 = P("id96", [96, 96], F32, isOutput=False)
    d_id128b = P("id128b", [128, 128], BF16, isOutput=False)
    d_lsi = P("lsidx", [128, 308], I16, isOutput=False)
    d_c1p = P("c1p", [114, 384], BF16, isOutput=False)
    d_c1s = P("c1s", [50, 384], BF16, isOutput=False)
    d_c2 = P("c2", [128, 576], BF16, isOutput=False)
    d_c3p = P("c3p", [128, 96], BF16, isOutput=False)
    d_c3s = P("c3s", [64, 96], BF16, isOutput=False)
    d_c4q = P("c4q", [128, 10], BF16, isOutput=False)
    d_c4s = P("c4s", [32, 10], BF16, isOutput=False)
    d_c4f = P("c4f", [3, 2], F32, isOutput=False)
    d_b1 = P("b1", [128, 1], F32, isOutput=False)
    d_b2 = P("b2", [64, 1], F32, isOutput=False)
    d_b3 = P("b3", [32, 1], F32, isOutput=False)
    d_al128 = P("al128", [128, 1], F32, isOutput=False)
    d_al64 = P("al64", [64, 1], F32, isOutput=False)
    d_al32 = P("al32", [32, 1], F32, isOutput=False)
    d_out = P("out", [2, H, W], F32, isOutput=True)
    d_wsc = nc.dram_tensor("wscratch", [4, HW], BF16)
    dbg = {}
    if debug:
        dbg["warped"] = P("dbg_warped", [96, WPH, WPW], BF16, isOutput=True)
        dbg["corr2"] = P("dbg_corr2", [114, CPH, CPW], BF16, isOutput=True)
        dbg["h1"] = P("dbg_h1", [128, CPH, CPW], BF16, isOutput=True)
        dbg["h3"] = P("dbg_h3", [128, QPH, QPW], BF16, isOutput=True)
        dbg["flow"] = P("dbg_flow", [2, H, W], F32, isOutput=True)
        dbg["il"] = P("dbg_il", [128, 960], I16, isOutput=True)

    with tile.TileContext(nc) as tc, ExitStack() as top:
        pc = top.enter_context(tc.tile_pool(name="pc", bufs=1))

        # ---- consts ----
        t_upw = pc.tile([48, 32], F32)
        t_id96 = pc.tile([96, 96], F32)
        t_id128b = pc.tile([128, 128], BF16)
        t_lsi = pc.tile([128, 308], I16)
        t_c1p = pc.tile([114, 384], BF16)
        t_c1s = pc.tile([50, 384], BF16)
        t_c2 = pc.tile([128, 576], BF16)
        t_c3p = pc.tile([128, 96], BF16)
        t_c3s = pc.tile([64, 96], BF16)
        t_c4q = pc.tile([128, 10], BF16)
        t_c4s = pc.tile([32, 10], BF16)
        t_c4f = pc.tile([3, 2], F32)
        t_b1 = pc.tile([128, 1], F32)
        t_b2 = pc.tile([64, 1], F32)
        t_b3 = pc.tile([32, 1], F32)
        t_al128 = pc.tile([128, 1], F32)
        t_al64 = pc.tile([64, 1], F32)
        t_al32 = pc.tile([32, 1], F32)
        for tt, dd in ((t_upw, d_upw),
                       (t_id96, d_id96), (t_id128b, d_id128b),
                       (t_lsi, d_lsi), (t_c1p, d_c1p), (t_c1s, d_c1s),
                       (t_c2, d_c2), (t_c3p, d_c3p), (t_c3s, d_c3s),
                       (t_c4q, d_c4q), (t_c4s, d_c4s), (t_c4f, d_c4f),
                       (t_b1, d_b1),
                       (t_b2, d_b2), (t_b3, d_b3),
                       (t_al128, d_al128), (t_al64, d_al64),
                       (t_al32, d_al32)):
            nc.sync.dma_start(tt[:], dd[:])

        # ---- upflow ----
        pp_cm = tc.tile_pool(name="pp", bufs=1)
        pp = pp_cm.__enter__()
        t_xg = pp.tile([H, W], F32)
        t_yg = pp.tile([H, W], F32)
        nc.sync.dma_start(t_xg[:], d_xg[:])
        nc.sync.dma_start(t_yg[:], d_yg[:])
        # fps[g][di+1]: flow_prev[g, p+di, q+dj] readable at col offset dj+1
        fps = {}
        for g in range(2):
            for di in (-1, 0, 1):
                nm = f"fps{g}_{di + 1}"
                t = pp.tile([48, 82], F32, tag=nm, name=nm)
                nc.vector.memset(t[:], 0.0)
                lo, hi = max(0, di), min(48, 48 + di)
                nc.sync.dma_start(t[lo - di:hi - di, 1:81], d_fp[g, lo:hi, :])
                fps[(g, di)] = t
        t_upt = pp.tile([48, 80], F32, tag="uptmp")
        planes = {}
        for key in {(g, ry, rx) for (g, ry, rx, _, _) in UP_ORDER}:
            nm = f"pl{key[0]}{key[1]}{key[2]}"
            planes[key] = pp.tile([48, 80], F32, tag=nm, name=nm)
        done = set()
        for j, (g, ry, rx, di, dj) in enumerate(UP_ORDER):
            pl = planes[(g, ry, rx)]
            sc = t_upw[:, j:j + 1]
            src = fps[(g, di)][:, 1 + dj:81 + dj]
            if (g, ry, rx) not in done:
                done.add((g, ry, rx))
                nc.vector.tensor_scalar(pl[:], src, sc, None, ALU.mult)
            else:
                nc.vector.tensor_scalar(t_upt[:], src, sc, None, ALU.mult)
                nc.vector.tensor_tensor(pl[:], pl[:], t_upt[:], ALU.add)
        t_flx = pc.tile([H, W], F32)
        t_fly = pc.tile([H, W], F32)
        flyx = [t_flx, t_fly]
        for (g, ry, rx), pl in sorted(planes.items()):
            nc.sync.dma_start(flyx[g][ry::2, rx::2], pl[:])

        # ---- warp index / weight pipeline ----
        def hwt(tag, dt=F32):
            return pp.tile([H, W], dt, tag=tag, name=tag)

        t_px, t_py = hwt("px"), hwt("py")
        nc.vector.tensor_scalar(t_px[:], t_flx[:], AX, None, ALU.mult)
        nc.vector.tensor_tensor(t_px[:], t_px[:], t_xg[:], ALU.add)
        nc.vector.tensor_scalar(t_py[:], t_fly[:], AY, None, ALU.mult)
        nc.vector.tensor_tensor(t_py[:], t_py[:], t_yg[:], ALU.add)
        t_x0, t_y0 = hwt("x0"), hwt("y0")
        t_i32 = pp.tile([H, W], I32, tag="i32")
        nc.vector.tensor_scalar(t_x0[:], t_px[:], EPSF, None, ALU.add)
        nc.vector.tensor_copy(t_i32[:], t_x0[:])
        nc.vector.tensor_copy(t_x0[:], t_i32[:])
        nc.vector.tensor_scalar(t_y0[:], t_py[:], EPSF, None, ALU.add)
        nc.vector.tensor_copy(t_i32[:], t_y0[:])
        nc.vector.tensor_copy(t_y0[:], t_i32[:])
        t_wx1, t_wy1, t_wx0, t_wy0 = (hwt("wx1"), hwt("wy1"),
                                      hwt("wx0"), hwt("wy0"))
        nc.vector.tensor_tensor(t_wx1[:], t_px[:], t_x0[:], ALU.subtract)
        nc.vector.tensor_tensor(t_wy1[:], t_py[:], t_y0[:], ALU.subtract)
        nc.vector.tensor_scalar(t_wx0[:], t_wx1[:], -1.0, 1.0, ALU.mult, ALU.add)
        nc.vector.tensor_scalar(t_wy0[:], t_wy1[:], -1.0, 1.0, ALU.mult, ALU.add)
        t_m1, t_m2 = hwt("m1"), hwt("m2")
        wviews = {}
        for nm, t_base, t_w, lo, hi in (
                ("wx0", t_x0, t_wx0, 0.0, float(W - 1)),
                ("wx1", t_x0, t_wx1, -1.0, float(W - 2)),
                ("wy0", t_y0, t_wy0, 0.0, float(H - 1)),
                ("wy1", t_y0, t_wy1, -1.0, float(H - 2))):
            nc.vector.tensor_scalar(t_m1[:], t_base[:], lo, None, ALU.is_ge)
            nc.vector.tensor_scalar(t_m2[:], t_base[:], hi, None, ALU.is_le)
            nc.vector.tensor_tensor(t_m1[:], t_m1[:], t_m2[:], ALU.mult)
            wv = pc.tile([H, W], BF16, tag=f"wv{nm}")
            nc.vector.tensor_tensor(wv[:], t_w[:], t_m1[:], ALU.mult)
            wviews[nm] = wv
        # fuse the x/y weights into the 4 per-tap products w_ij =
        # wx_j * wy_i (kills the separate y-multiply stage in the warp
        # combine), then flatten each [H,W] plane to a d_wsc row
        wprod = {}
        for wi, (nm, nx, ny) in enumerate(
                (("w00", "wx0", "wy0"), ("w01", "wx1", "wy0"),
                 ("w10", "wx0", "wy1"), ("w11", "wx1", "wy1"))):
            wp = pc.tile([H, W], BF16, tag=f"wp{nm}")
            nc.vector.tensor_tensor(wp[:], wviews[nx][:], wviews[ny][:],
                                    ALU.mult)
            wprod[nm] = wp
            nc.sync.dma_start(d_wsc[wi:wi + 1, :], wp[:])
        t_idx = hwt("idxf")
        nc.vector.tensor_scalar(t_m1[:], t_y0[:], -1.0, float(H - 1),
                                ALU.max, ALU.min)
        nc.vector.tensor_scalar(t_m2[:], t_x0[:], -1.0, float(W - 1),
                                ALU.max, ALU.min)
        nc.vector.tensor_scalar(t_idx[:], t_m1[:], float(W), float(PAD),
                                ALU.mult, ALU.add)
        nc.vector.tensor_tensor(t_idx[:], t_idx[:], t_m2[:], ALU.add)

        # ---- interleave idx -> il [96, 960] int16 ----
        t_il = pc.tile([128, 960], I16)
        with tc.tile_pool(name="ps_tr", bufs=2,
                          space=bass.MemorySpace.PSUM) as ps_tr:
            for u in range(10):
                p_t = ps_tr.tile([16, 96], F32, tag="pt", name="p_t")
                nc.tensor.transpose(p_t[:], t_idx[:, 16 * u:16 * u + 16],
                                    t_id96[:])
                nc.vector.tensor_copy(t_il[0:16, u::10], p_t[:])
        k = 16
        while k < 128:
            n = min(k, 128 - k)
            nc.sync.dma_start(t_il[k:k + n, :], t_il[0:n, :])
            k += n
        if debug:
            nc.sync.dma_start(dbg["il"][:], t_il[:])
        pp_cm.__exit__(None, None, None)

        pcv = top.enter_context(tc.tile_pool(name="pcv", bufs=1))
        t_corr2 = pcv.tile([114, CPH, CPW], BF16)

        # ---- warp phase ----
        pmA_cm = tc.tile_pool(name="pmA", bufs=1)
        pmA = pmA_cm.__enter__()
        t_warp = pmA.tile([96, WPH, WPW], BF16)
        nc.vector.memset(t_warp[:, 0:3, :], 0.0)
        nc.vector.memset(t_warp[:, WPH - 3:WPH, :], 0.0)
        nc.vector.memset(t_warp[:, 3:WPH - 3, 0:3], 0.0)
        nc.vector.memset(t_warp[:, 3:WPH - 3, WPW - 3:WPW], 0.0)

        # per quarter: 2 paired gathers x 5 chunks (elem 256 @ step 128
        # fetches the x/x+1 taps together), 4 weight-product broadcasts,
        # in-place bilinear combine. bufs=2 pipelines quarters.
        with tc.tile_pool(name="pw", bufs=2) as pw, \
                tc.tile_pool(name="pg", bufs=2) as pg:
            for q in range(4):
                r0 = q * 24
                reps = {}
                for wi, nm in enumerate(("w00", "w01", "w10", "w11")):
                    rep = pw.tile([96, QP], BF16, tag=f"rp{nm}", name=nm)
                    src = d_wsc[wi:wi + 1, q * QP:(q + 1) * QP]
                    rsrc = bass.AP(src.tensor, src.offset,
                                   [[0, 96]] + list(src.ap)[1:])
                    nc.sync.dma_start(rep[:], rsrc)
                    reps[nm] = rep
                gA = pg.tile([128, 6, 2, 640], BF16, tag="gA")
                gB = pg.tile([128, 6, 2, 640], BF16, tag="gB")
                for c6 in range(6):
                    for ti, (gt, off) in enumerate(((gA, 0), (gB, W))):
                        src = bass.AP(d_f2t[:].tensor,
                                      d_f2t[:].offset + off * 128,
                                      [[128, NEG], [1, 256]])
                        nc.gpsimd.dma_gather(
                            gt[:, c6, :, :], src,
                            t_il[:, 240 * q + 40 * c6:240 * q + 40 * (c6 + 1)],
                            num_idxs=640, num_idxs_reg=640, elem_size=256,
                            elem_step=128, transpose=True,
                            # single queue for ALL gathers: lane sems are
                            # cumulative, so mixing queues on one lane lets
                            # a later gather satisfy an earlier wait
                            # threshold out of order -> data races. One
                            # queue = global FIFO = sound thresholds.
                            queue_num=0)
                # chunk = 640 px = exactly 4 warp rows, so the strided
                # [96, 6, 4, 160] views line up with the padded t_warp
                gA0 = gA[0:96, :, 0, :]
                gA1 = gA[0:96, :, 1, :]
                gB0 = gB[0:96, :, 0, :]
                gB1 = gB[0:96, :, 1, :]
                rw = {nm: reps[nm][:].rearrange("p (c n) -> p c n", c=6)
                      for nm in reps}
                nc.vector.tensor_tensor(gA0, gA0, rw["w00"], ALU.mult)
                nc.vector.tensor_tensor(gA1, gA1, rw["w01"], ALU.mult)
                nc.vector.tensor_tensor(gA0, gA0, gA1, ALU.add)
                nc.vector.tensor_tensor(gB0, gB0, rw["w10"], ALU.mult)
                nc.vector.tensor_tensor(gB1, gB1, rw["w11"], ALU.mult)
                nc.vector.tensor_tensor(gB0, gB0, gB1, ALU.add)
                wdst = t_warp[:, 3 + r0:3 + r0 + 24, 3:3 + W]
                wdst = wdst.rearrange("p (c r) w -> p c r w", r=4)
                nc.vector.tensor_tensor(
                    wdst, gA0.rearrange("p c (r w) -> p c r w", w=W),
                    gB0.rearrange("p c (r w) -> p c r w", w=W), ALU.add)
                # WAR guard: the strided final-add read above is missed by
                # the slot reuse tracking; these DVE writes are engine-
                # ordered after it, so next quarter's gathers (WAW) can't
                # overwrite gA/gB while it still reads them
                nc.vector.memset(gA[0:32, 0, 0, 0:2], 0.0)
                nc.vector.memset(gB[0:32, 0, 0, 0:2], 0.0)
                for nm, rep in reps.items():
                    nc.vector.memset(rep[0:32, 0:2], 0.0)

        if debug:
            nc.sync.dma_start(dbg["warped"][:], t_warp[:])
            nc.sync.dma_start(dbg["flow"][0], t_flx[:])
            nc.sync.dma_start(dbg["flow"][1], t_fly[:])

        # ---- correlation ----
        nc.vector.memset(t_corr2[32:64], 0.0)
        nc.vector.memset(t_corr2[:, 0:1, :], 0.0)
        nc.vector.memset(t_corr2[:, CPH - 1:CPH, :], 0.0)
        nc.vector.memset(t_corr2[:, 1:CPH - 1, 0:1], 0.0)
        nc.vector.memset(t_corr2[:, 1:CPH - 1, W + 1:CPW], 0.0)

        pf1_cm = tc.tile_pool(name="pf1", bufs=1)
        pf1 = pf1_cm.__enter__()
        t_f1b = pf1.tile([96, HW], BF16)
        # band-wise loads: band b's matmuls only wait on chunk b
        for b6 in range(6):
            nc.sync.dma_start(t_f1b[:, b6 * 2560:(b6 + 1) * 2560],
                              d_f1b[:, b6 * 2560:(b6 + 1) * 2560])

        with tc.tile_pool(name="pcr", bufs=4) as pcr, \
                tc.tile_pool(name="pst", bufs=4) as pst, \
                tc.tile_pool(name="ps_c", bufs=4,
                             space=bass.MemorySpace.PSUM) as ps_c, \
                tc.tile_pool(name="ps_p", bufs=2,
                             space=bass.MemorySpace.PSUM) as ps_p:
            for band in range(6):
                Y = band * 16
                for grp in range(3):
                    npair = 4 if grp < 2 else 2
                    p_pa = ps_p.tile([50, 512], BF16, tag="packa")
                    p_pb = ps_p.tile([50, 512], BF16, tag="packb")
                    for pj in range(npair):
                        stk = pst.tile([128, 100], BF16, tag="stk")
                        for half in range(2):
                            tx = grp * 8 + pj * 2 + half
                            X = tx * 8
                            p_c = ps_c.tile([128, 308], F32, tag="pcorr")
                            ti128 = (band * 20 + tx) * 128
                            nc.tensor.matmul(
                                p_c[:], t_f1b[:, ti128:ti128 + 128],
                                t_warp[:, Y:Y + 22, X:X + 14],
                                start=True, stop=True)
                            sb = pcr.tile([128, 308], BF16, tag="sbc")
                            if half == 0:
                                nc.vector.tensor_copy(sb[:], p_c[:])
                            else:
                                nc.scalar.activation(sb[:], p_c[:], ACTF.Copy)
                            nc.gpsimd.local_scatter(
                                stk[:, half * 50:half * 50 + 50], sb[:],
                                t_lsi[:], channels=128, num_elems=50,
                                num_idxs=308)
                        nc.tensor.transpose(
                            p_pa[:, pj * 128:(pj + 1) * 128], stk[:, 0:50],
                            t_id128b[:])
                        nc.tensor.transpose(
                            p_pb[:, pj * 128:(pj + 1) * 128], stk[:, 50:100],
                            t_id128b[:])
                    for half in range(2):
                        xbase = grp * 64 + half * 8
                        cv = t_corr2[:]
                        shp = [[CPH * CPW, 50], [16, npair], [CPW, 16], [1, 8]]
                        dst = bass.AP(
                            cv.tensor,
                            cv.offset + (1 + Y) * CPW + (1 + xbase), shp)
                        # kx1-tap duplicate: rows 64:114, cols shifted by -1
                        dst2 = bass.AP(
                            cv.tensor,
                            cv.offset + 64 * (CPH * CPW) + (1 + Y) * CPW
                            + xbase, shp)
                        src = (p_pa if half == 0 else p_pb)[:]
                        src = src.rearrange("p (j r c) -> p j r c", r=16, c=8)
                        src = src[:, 0:npair]
                        # fused lrelu on copy-out (ACT), then partition-
                        # shifted duplicate for the conv1 kx1 K-pack (DVE)
                        nc.scalar.activation(dst, src, ACTF.Prelu,
                                             bias=0.0, scale=1.0,
                                             alpha=t_al128[0:50])
                        nc.vector.tensor_copy(dst2, dst)

        pf1_cm.__exit__(None, None, None)
        pmA_cm.__exit__(None, None, None)

        if debug:
            nc.sync.dma_start(dbg["corr2"][:], t_corr2[:])

        # ---- convs ----
        pcv2 = top.enter_context(tc.tile_pool(name="pcv2", bufs=1))
        t_h1 = pcv2.tile([128, CPH, CPW], BF16)
        nc.vector.memset(t_h1[:, 0:1, :], 0.0)
        nc.vector.memset(t_h1[:, CPH - 1:CPH, :], 0.0)
        nc.vector.memset(t_h1[:, 1:CPH - 1, 0:1], 0.0)
        nc.vector.memset(t_h1[:, 1:CPH - 1, W + 1:CPW], 0.0)

        with tc.tile_pool(name="ps_cv", bufs=4,
                          space=bass.MemorySpace.PSUM) as ps_cv:
            # conv1
            for ch in range(NCH):
                r = 3 * ch
                p_o = ps_cv.tile([128, CHP], F32, tag="cvo")
                for ky in range(3):
                    nc.tensor.matmul(
                        p_o[:], t_c1p[:, ky * 128:(ky + 1) * 128],
                        t_corr2[0:114, r + ky:r + ky + 3, 0:W],
                        start=(ky == 0), stop=False)
                    nc.tensor.matmul(
                        p_o[:], t_c1s[:, ky * 128:(ky + 1) * 128],
                        t_corr2[0:50, r + ky:r + ky + 3, 2:2 + W],
                        start=False, stop=(ky == 2))
                nc.scalar.activation(
                    t_h1[:, r + 1:r + 4, 1:1 + W],
                    p_o[:].rearrange("p (r w) -> p r w", w=W),
                    ACTF.Prelu, bias=t_b1[:], scale=1.0, alpha=t_al128[:])
            if debug:
                nc.sync.dma_start(dbg["h1"][:], t_h1[:])

            # conv2 — chunk pairs run concurrently on the two 64-col PE
            # tiles (tile_position derives from the psum partition base)
            t_h2 = pcv2.tile([128, CPH, CPW], BF16)
            nc.vector.memset(t_h2[:, 0:1, :], 0.0)
            nc.vector.memset(t_h2[:, CPH - 1:CPH, :], 0.0)
            nc.vector.memset(t_h2[:, 1:CPH - 1, 0:1], 0.0)
            nc.vector.memset(t_h2[:, 1:CPH - 1, W + 1:CPW], 0.0)
            for pch in range(0, NCH, 2):
                p_o = ps_cv.tile([128, CHP], F32, tag="cvo")
                for ti in range(9):
                    ky, kx = divmod(ti, 3)
                    for half in range(2):
                        r = 3 * (pch + half)
                        nc.tensor.matmul(
                            p_o[64 * half:64 * half + 64],
                            t_c2[:, ti * 64:(ti + 1) * 64],
                            t_h1[:, r + ky:r + ky + 3, kx:kx + W],
                            start=(ti == 0), stop=(ti == 8),
                            skip_group_check=True,
                            tile_position=(0, 64 * half))
                for half in range(2):
                    r = 3 * (pch + half)
                    nc.scalar.activation(
                        t_h2[0:64, r + 1:r + 4, 1:1 + W],
                        p_o[64 * half:64 * half + 64].rearrange(
                            "p (r w) -> p r w", w=W),
                        ACTF.Prelu, bias=t_b2[:], scale=1.0, alpha=t_al64[:])
                    # kx1-tap duplicate for conv3 K-pack: partitions
                    # 64:128, cols shifted by -1 (DVE, overlaps matmuls)
                    nc.vector.tensor_copy(
                        t_h2[64:128, r + 1:r + 4, 0:W],
                        t_h2[0:64, r + 1:r + 4, 1:1 + W])

            # conv3 -> h3 (padded 100x165 @ (2,2)); the 3 row-shifted
            # ky planes for conv4's K-pack are built chunk-by-chunk with
            # partition-shifted engine copies that overlap the matmuls
            t_h3 = pcv2.tile([128, QPH, QPW], BF16)
            nc.vector.memset(t_h3[0:32, 0:2, :], 0.0)
            nc.vector.memset(t_h3[0:32, QPH - 2:QPH, :], 0.0)
            nc.vector.memset(t_h3[:, 2:QPH - 2, 0:2], 0.0)
            nc.vector.memset(t_h3[:, 2:QPH - 2, W + 2:QPW], 0.0)
            nc.vector.memset(t_h3[32:64, 0:2, :], 0.0)
            nc.vector.memset(t_h3[64:128, 0:2, :], 0.0)
            nc.vector.memset(t_h3[96:128, 95:96, :], 0.0)
            # chunk quads run concurrently on the four 32-col PE tiles
            for qch in range(0, NCH, 4):
                p_o = ps_cv.tile([128, CHP], F32, tag="cvo")
                for ky in range(3):
                    for m in range(4):
                        r = 3 * (qch + m)
                        nc.tensor.matmul(
                            p_o[32 * m:32 * m + 32],
                            t_c3p[:, ky * 32:(ky + 1) * 32],
                            t_h2[0:128, r + ky:r + ky + 3, 0:W],
                            start=(ky == 0), stop=False,
                            skip_group_check=True,
                            tile_position=(0, 32 * m))
                    for m in range(4):
                        r = 3 * (qch + m)
                        nc.tensor.matmul(
                            p_o[32 * m:32 * m + 32],
                            t_c3s[:, ky * 32:(ky + 1) * 32],
                            t_h2[0:64, r + ky:r + ky + 3, 2:2 + W],
                            start=False, stop=(ky == 2),
                            skip_group_check=True,
                            tile_position=(0, 32 * m))
                for m in range(4):
                    r = 3 * (qch + m)
                    nc.scalar.activation(
                        t_h3[0:32, r + 2:r + 5, 2:2 + W],
                        p_o[32 * m:32 * m + 32].rearrange(
                            "p (r w) -> p r w", w=W),
                        ACTF.Prelu, bias=t_b3[:], scale=1.0, alpha=t_al32[:])
                    for dr, eng in ((1, nc.vector), (2, nc.gpsimd),
                                    (3, nc.vector)):
                        lo = max(0, r + 2 - dr)
                        hi = r + 5 - dr
                        eng.tensor_copy(
                            t_h3[32 * dr:32 * dr + 32, lo:hi, 2:2 + W],
                            t_h3[0:32, lo + dr:hi + dr, 2:2 + W])
            if debug:
                nc.sync.dma_start(dbg["h3"][:], t_h3[:])

            # conv4: chunk quads on the four 32-col PE tiles; flow + bias
            # accumulated into PSUM via a K=3 identity/bias matmul (f32r)
            with tc.tile_pool(name="po4", bufs=3) as po4:
                for qg in range(4):
                    t_fl3 = po4.tile([3, QP], F32, tag="flfq", bufs=2)
                    rq = qg * 24
                    # engine memsets must start at partition 0/32/64/96:
                    # fill all 3 rows with 1.0, then overwrite 0:2 w/ flow
                    nc.vector.memset(t_fl3[0:3, :], 1.0)
                    nc.sync.dma_start(t_fl3[0:1, :], t_flx[rq:rq + 24, :])
                    nc.sync.dma_start(t_fl3[1:2, :], t_fly[rq:rq + 24, :])
                    t_oq = po4.tile([2, QP], F32, tag="oq", bufs=2)
                    for qc in range(0, 8, 4):
                        p_o = ps_cv.tile([128, CHP], F32, tag="cvo")
                        for kx in range(5):
                            for m in range(4):
                                r = 3 * (qg * 8 + qc + m)
                                nc.tensor.matmul(
                                    p_o[32 * m:32 * m + 2],
                                    t_c4q[:, kx * 2:kx * 2 + 2],
                                    t_h3[0:128, r:r + 3, kx:kx + W],
                                    start=(kx == 0), stop=False,
                                    skip_group_check=True,
                                    tile_position=(0, 32 * m))
                            for m in range(4):
                                r = 3 * (qg * 8 + qc + m)
                                nc.tensor.matmul(
                                    p_o[32 * m:32 * m + 2],
                                    t_c4s[:, kx * 2:kx * 2 + 2],
                                    t_h3[0:32, r + 4:r + 7, kx:kx + W],
                                    start=False, stop=False,
                                    skip_group_check=True,
                                    tile_position=(0, 32 * m))
                        for m in range(4):
                            cc = qc + m
                            nc.tensor.matmul(
                                p_o[32 * m:32 * m + 2],
                                t_c4f[:],
                                t_fl3[:, cc * CHP:(cc + 1) * CHP],
                                start=False, stop=True,
                                skip_group_check=True,
                                tile_position=(0, 32 * m))
                        for m in range(4):
                            cc = qc + m
                            nc.vector.tensor_copy(
                                t_oq[:, cc * CHP:(cc + 1) * CHP],
                                p_o[32 * m:32 * m + 2])
                    nc.sync.dma_start(
                        d_out[:, rq:rq + 24, :],
                        t_oq[:].rearrange("p (r w) -> p r w", w=W))

    nc.compile()
    return nc


_STATE = {}


def _make_runner(nc):
    """Build a persistent jitted shard_map callable for the compiled Bass
    module (mirrors bass2jax.run_bass_via_pjrt, but reusable + exposes
    device placement for steady-state timing)."""
    import jax
    import numpy as _np
    from jax.sharding import Mesh, PartitionSpec, NamedSharding
    from jax.experimental.shard_map import shard_map
    from concourse import bass2jax as b2j
    from concourse import mybir as _mb

    b2j.install_neuronx_cc_hook()
    partition_name = (nc.partition_id_tensor.name
                      if nc.partition_id_tensor else None)
    in_names, out_names, out_avals, zero_outs = [], [], [], []
    for alloc in nc.m.functions[0].allocations:
        if not isinstance(alloc, _mb.MemoryLocationSet):
            continue
        name = alloc.memorylocations[0].name
        if alloc.kind == "ExternalInput":
            if name != partition_name:
                in_names.append(name)
        elif alloc.kind == "ExternalOutput":
            shape = tuple(alloc.tensor_shape)
            dtype = _mb.dt.np(alloc.dtype)
            out_names.append(name)
            out_avals.append(jax.core.ShapedArray(shape, dtype))
            zero_outs.append(_np.zeros(shape, dtype))
    n_params = len(in_names)
    all_in = list(in_names) + list(out_names)
    if partition_name is not None:
        all_in.append(partition_name)

    def _body(*args):
        operands = list(args)
        if partition_name is not None:
            operands.append(b2j.partition_id_tensor())
        outs = b2j._bass_exec_p.bind(
            *operands,
            out_avals=tuple(out_avals),
            in_names=tuple(all_in),
            out_names=tuple(out_names),
            lowering_input_output_aliases=(),
            sim_require_finite=True,
            sim_require_nnan=True,
            nc=nc,
        )
        return tuple(outs)

    devices = jax.devices()[:N_CORES]
    mesh = Mesh(np.asarray(devices), ("core",))
    nsh = len(in_names) + len(out_names)
    sharded = jax.jit(
        shard_map(_body, mesh=mesh,
                  in_specs=(PartitionSpec("core"),) * nsh,
                  out_specs=(PartitionSpec("core"),) * len(out_names),
                  check_rep=False),
        keep_unused=True)
    sharding = NamedSharding(mesh, PartitionSpec("core"))
    return {
        "in_names": in_names, "out_names": out_names,
        "zero_outs": zero_outs, "sharded": sharded, "sharding": sharding,
        "out_avals": out_avals,
    }


def _get_state(debug=False):
    key = "dbg" if debug else "main"
    if key not in _STATE:
        nc = build_program(debug=debug)
        _STATE[key] = {"nc": nc, "consts": _host_consts(),
                       "runner": _make_runner(nc)}
    return _STATE[key]


def _build_in_maps(feat_one, feat_two, flow_prev, up_w,
                   w1, b1, w2, b2, w3, b3, w4, b4, consts):
    ws = _host_weights(np.asarray(up_w, np.float32),
                       np.asarray(w1, np.float32), np.asarray(b1, np.float32),
                       np.asarray(w2, np.float32), np.asarray(b2, np.float32),
                       np.asarray(w3, np.float32), np.asarray(b3, np.float32),
                       np.asarray(w4, np.float32), np.asarray(b4, np.float32))
    shared = {"xg": consts["xg"], "yg": consts["yg"], "id96": consts["id96"],
              "id128b": consts["id128b"], "lsidx": consts["lsidx"]}
    for nm in ("upwtab", "c1p", "c1s", "c2", "c3p", "c3s", "c4q", "c4s",
               "c4f", "b1", "b2", "b3", "al128", "al64", "al32"):
        shared[nm] = ws[nm]
    f1 = np.asarray(feat_one, np.float32).reshape(B, 96, HW)
    f2 = np.asarray(feat_two, np.float32).reshape(B, 96, HW)
    fp = np.asarray(flow_prev, np.float32)
    in_maps = []
    for i in range(N_CORES):
        m = dict(shared)
        f1t = (f1[i] * (1.0 / 96.0)).reshape(96, 6, 16, 20, 8)
        m["f1b"] = np.ascontiguousarray(
            f1t.transpose(0, 1, 3, 2, 4)).reshape(96, HW).astype(bf)
        ft = np.zeros((F2LEN, 128), bf)
        ft[PAD:PAD + HW, 0:96] = f2[i].T
        m["f2t"] = ft
        m["fp"] = fp[i]
        in_maps.append(m)
    return in_maps


def stage_inputs(in_maps, runner):
    """Concatenate per-core inputs on axis 0 and place on the 8 cores."""
    import jax
    args = []
    for nm in runner["in_names"]:
        args.append(np.concatenate([np.asarray(m[nm]) for m in in_maps],
                                   axis=0))
    for z in runner["zero_outs"]:
        args.append(np.zeros((N_CORES * z.shape[0], *z.shape[1:]), z.dtype))
    return [jax.device_put(a, runner["sharding"]) for a in args]


def run_staged(runner, dev_args):
    return runner["sharded"](*dev_args)


def kernel(feat_one, feat_two, flow_prev, up_w,
           w1, b1, w2, b2, w3, b3, w4, b4, debug=False):
    st = _get_state(debug)
    runner = st["runner"]
    in_maps = _build_in_maps(feat_one, feat_two, flow_prev, up_w,
                             w1, b1, w2, b2, w3, b3, w4, b4, st["consts"])
    dev_args = stage_inputs(in_maps, runner)
    outs = run_staged(runner, dev_args)
    oi = runner["out_names"].index("out")
    out = np.asarray(outs[oi]).reshape(N_CORES, 2, H, W).astype(np.float32)
    if debug:
        results = []
        for i in range(N_CORES):
            r = {}
            for j, nm in enumerate(runner["out_names"]):
                a = runner["out_avals"][j]
                r[nm] = np.asarray(outs[j]).reshape(N_CORES, *a.shape)[i]
            results.append(r)
        return out, results
    return out

